# revision 13
# baseline (speedup 1.0000x reference)
"""AGCRNCell distributed Bass kernel for 8 TRN2 NeuronCores.

Batch-parallel: B=16 -> 2 batches/core, zero collectives.  Each core:
  A = exp(relu(E @ E^T))      (symmetric -> A^T = A, no transposes;
                               softmax normalization deferred: S@v =
                               rinv * (A@v), rinv applied at eviction)
  diffusion hops as dense bf16 matmuls over 128x128 tiles of A,
  Chebyshev term folded into the weight pools host-side:
      sum_k xg_k w_k = xg0 (w0-w2) + y1 w1 + u2 (2 w2),
      y1 = rinv*(A@x), u2 = rinv*(A@y1)
  per-node adaptive weights factored through the embedding dim D=10:
      out[n,o] = sum_d E[n,d] * (xg[n,:] @ wpool[d,:,(o)])
  with weight pools laid out (o,d)-interleaved so the d-contraction is
  one DVE tensor_tensor(mult, E broadcast) + one tensor_reduce(X) per
  chunk, batched over both local batches.

v2: HAM warmup burst (PE idles ~90us at 1.2GHz otherwise), fused
1024-wide exp eviction via a shared 2-bank PSUM tag, weight phase
batched over b (half the DVE/scalar instructions), bf16 d-sum
accumulate, j-major OUT layout with streamed DMA.
"""

import numpy as np
import ml_dtypes

import concourse.bass as bass
import concourse.mybir as mybir
import concourse.tile as tile
from concourse import bacc
from concourse.bass_utils import run_bass_kernel_spmd
from concourse.masks import make_identity

BF = mybir.dt.bfloat16
F32 = mybir.dt.float32
F32R = mybir.dt.float32r

B, N, C, D, K = 16, 2048, 64, 10, 3
NCORES = 8
B2 = B // NCORES          # 2 batches per core
MT = N // 128             # 16 row tiles
C2 = 2 * C                # 128
OG = 2 * C                # gate output width 128
OU = C                    # update output width 64
WOG = OG * D              # 1280 (o,d)-interleaved gate width
WOU = OU * D              # 640
GCH = [(0, 510), (510, 510), (1020, 260)]   # gate (o,d) chunks, mult of 10
UCH = [(0, 510), (510, 130)]                # update chunks

_CACHE = {}


def _build():
    nc = bacc.Bacc("TRN2", target_bir_lowering=False, debug=False,
                   num_devices=NCORES)

    def inp(name, shape, dt):
        return nc.dram_tensor(name, list(shape), dt, kind="ExternalInput").ap()

    xc_d = inp("xc", (128, MT * B2 * C2), BF)    # [p, (m,b,c2)] xcat tiles
    ew_d = inp("ew", (D, N), F32R)               # E^T
    et_d = inp("et", (128, MT * D), F32)         # [p, (j,d)] E rows
    gw_d = inp("gw", (128, K * WOG), BF)         # [c2, (k,o,d)] gate pool
    uw_d = inp("uw", (128, K * WOU), BF)
    gb_d = inp("gb", (128, MT * OG), BF)         # [p, (j,o)] E@gate_bpool
    ub_d = inp("ub", (128, MT * OU), BF)
    lw_d = inp("lw", (C2, C), BF)                # lin_w^T
    lb_d = inp("lb", (128, C), F32)              # lin_b tiled
    out_d = nc.dram_tensor("out", [128, MT * B2 * C], F32,
                           kind="ExternalOutput").ap()   # [p, (j,b,c)]

    AFT = mybir.ActivationFunctionType
    MULT = mybir.AluOpType.mult
    ADD = mybir.AluOpType.add
    AXX = mybir.AxisListType.X

    with tile.TileContext(nc) as tc:
        with (
            tc.tile_pool(name="const", bufs=1) as const,
            tc.tile_pool(name="sraw", bufs=2) as srp,
            tc.tile_pool(name="stat", bufs=4) as stat,
            tc.tile_pool(name="xgp", bufs=4) as xgp,
            tc.tile_pool(name="scp", bufs=4) as scp,
            tc.tile_pool(name="accp", bufs=4) as accp,
            tc.tile_pool(name="sgp", bufs=4) as sgp,
            # PSUM: "big" 2 banks x3, "pd" 1x2 = 8 banks
            tc.tile_pool(name="pB", bufs=3, space="PSUM") as pB,
            tc.tile_pool(name="pD", bufs=2, space="PSUM") as pD,
        ):
            ident = const.tile([128, 128], BF)
            make_identity(nc, ident[:])

            # ---- HAM warmup: dense matmuls so the PE clock un-throttles
            # (K=4/8 -> 8/8) and stays there while the input DMAs land.
            # Small tensors (EW for EE^T) are DMA'd first so real PE work
            # starts within ~2us.
            wrm = const.tile([128, 512], BF)
            nc.gpsimd.memset(wrm[:], 0.0)

            EW = const.tile([D, N], F32R)
            nc.sync.dma_start(EW[:], ew_d[:])
            ET = const.tile([128, MT * D], F32)
            nc.sync.dma_start(ET[:], et_d[:])
            LW = const.tile([C2, C], BF)
            nc.sync.dma_start(LW[:], lw_d[:])
            LB = const.tile([128, C], F32)
            nc.sync.dma_start(LB[:], lb_d[:])
            GB = const.tile([128, MT * OG], BF)
            nc.sync.dma_start(GB[:], gb_d[:])
            UB = const.tile([128, MT * OU], BF)
            nc.sync.dma_start(UB[:], ub_d[:])
            XC = const.tile([128, MT * B2 * C2], BF)
            nc.sync.dma_start(XC[:], xc_d[:])
            GW = const.tile([128, K * WOG], BF)
            nc.sync.dma_start(GW[:], gw_d[:])
            UW = const.tile([128, K * WOU], BF)
            nc.sync.dma_start(UW[:], uw_d[:])

            for i in range(10):
                pwm = pB.tile([128, 1024], F32, tag="big", name="pwm")
                nc.tensor.matmul(pwm[:, 0:512], ident[:], wrm[:],
                                 start=True, stop=True)

            A = [const.tile([128, N], BF, tag=f"A{j}", name=f"A{j}")
                 for j in range(MT)]
            RINV = const.tile([128, MT], F32)
            OUT = const.tile([128, MT * B2 * C], F32)   # [p, (j,b,c)]

            # persistent diffusion state
            Y1 = const.tile([128, MT * B2 * C2], BF)    # rinv*(A@[x|s])
            U2 = const.tile([128, MT * B2 * C2], BF)    # rinv*(A@Y1)
            # stitched [x|zr], [y1x|y1z], [u2x|u2z] per (j, b): 64+64 cols
            XZ = const.tile([128, MT * B2 * C2], BF)
            YZ = const.tile([128, MT * B2 * C2], BF)
            UZ = const.tile([128, MT * B2 * C2], BF)

            def cat3(t, j):
                return t[:, j * 256:(j + 1) * 256] \
                    .rearrange("p (b c) -> p b c", b=B2)

            # partial accumulators for the split first hop (m 0..7 during
            # phase S, m 8..15 after)
            PRT = const.tile([128, MT * B2 * C2], BF)

            def d1a_passA(j):
                pd = pD.tile([128, B2 * C2], F32, tag="pd")
                for m in range(MT // 2):
                    nc.tensor.matmul(pd[:],
                                     A[m][:, j * 128:(j + 1) * 128],
                                     XC[:, m * 256:(m + 1) * 256],
                                     start=(m == 0), stop=(m == MT // 2 - 1))
                nc.scalar.activation(PRT[:, j * 256:(j + 1) * 256], pd[:],
                                     AFT.Copy, scale=RINV[:, j:j + 1])

            # ---- phase S: A = max(exp(E@E^T), 1), rinv = 1/rowsum ----
            # (exp(relu(x)) == max(exp(x), 1); clamp+rowsum fused on DVE).
            # EE^T lands in 2-bank PSUM tiles so exp evicts 1024 wide.
            # Once the first 8 A-tiles exist, the first half of the A@[x|s]
            # hop runs interleaved to keep the PE busy under S's DVE chain.
            for j in range(MT):
                etmp = srp.tile([128, N], BF, tag="etmp")
                for h in range(2):
                    ps = pB.tile([128, 1024], F32, tag="big", name="ps")
                    for q in range(2):
                        nc.tensor.matmul(ps[:, q * 512:(q + 1) * 512],
                                         EW[:, j * 128:(j + 1) * 128],
                                         EW[:, (2 * h + q) * 512:
                                             (2 * h + q + 1) * 512],
                                         start=True, stop=True)
                    nc.scalar.activation(etmp[:, h * 1024:(h + 1) * 1024],
                                         ps[:], AFT.Exp)
                zs = stat.tile([128, 1], F32, tag="zs")
                nc.vector.tensor_scalar(A[j][:], etmp[:], 1.0, 0.0,
                                        mybir.AluOpType.max,
                                        mybir.AluOpType.add,
                                        accum_out=zs[:])
                nc.vector.reciprocal(RINV[:, j:j + 1], zs[:])
                if j >= MT // 2:
                    d1a_passA(j - MT // 2)
                # keep PE from idling a full HAM window while the
                # scalar/DVE chain drains (idle >3.4us re-throttles
                # the clock to 1.2GHz for the rest of phase S)
                pdw = pD.tile([128, B2 * C2], F32, tag="pd", name="pdw")
                for _ in range(4 if j < MT // 2 else 2):
                    nc.tensor.matmul(pdw[:], ident[:], wrm[:, 0:256],
                                     start=True, stop=True)
            for j in range(MT // 2, MT):
                d1a_passA(j)

            # ---- diffusion hop: dst_j = rinv_j * (A @ rhs) ----
            def hop_j(j, rhs_fn, evict_fn, m0=0):
                pd = pD.tile([128, B2 * C2], F32, tag="pd")
                w = rhs_fn(0).free_size()
                for m in range(m0, MT):
                    nc.tensor.matmul(pd[:, 0:w],
                                     A[m][:, j * 128:(j + 1) * 128],
                                     rhs_fn(m),
                                     start=(m == m0), stop=(m == MT - 1))
                evict_fn(pd[:, 0:w])

            def full_evict(dst, j):
                def ev(pdw):
                    nc.scalar.activation(dst[:, j * 256:(j + 1) * 256], pdw,
                                         AFT.Copy, scale=RINV[:, j:j + 1])
                return ev

            def z_evict(dst, j):
                # write z-halves into cols [64:128] of each 128-col group
                def ev(pdw):
                    nc.scalar.activation(
                        cat3(dst, j)[:, :, C:C2],
                        pdw.rearrange("p (b c) -> p b c", b=B2),
                        AFT.Copy, scale=RINV[:, j:j + 1])
                return ev

            for j in range(MT):
                # second half of the first hop; fused add of the pass-A
                # partial during eviction
                def evA(pdw, j=j):
                    nc.vector.scalar_tensor_tensor(
                        Y1[:, j * 256:(j + 1) * 256], pdw,
                        RINV[:, j:j + 1],
                        PRT[:, j * 256:(j + 1) * 256],
                        MULT, ADD)
                hop_j(j, lambda m: XC[:, m * 256:(m + 1) * 256], evA,
                      m0=MT // 2)
                # prefill x / y1x columns of the stitched tiles (gpsimd:
                # SBUF-to-SBUF, keeps scalar/DVE free)
                nc.gpsimd.tensor_copy(cat3(XZ, j)[:, :, 0:C],
                                      cat3(XC, j)[:, :, 0:C])
                nc.gpsimd.tensor_copy(cat3(YZ, j)[:, :, 0:C],
                                      cat3(Y1, j)[:, :, 0:C])

            # ---- weight application: prep (transposes) + main, software-
            # pipelined one j apart so the PE never heads-of-line blocks on
            # the cross-engine chain.  Both local batches are processed in
            # one batched PSUM tile / DVE op per chunk.
            def weight_prep(is_gate, j):
                # xg transposes on the DMA xbar (idle engine) instead of
                # PE+scalar: one batched 16x128-tiled transpose per source,
                # covering both local batches.
                srcs = (XC, Y1, U2) if is_gate else (XZ, YZ, UZ)
                eng = nc.sync if is_gate else nc.scalar
                xgT = xgp.tile([128, B2, K, 128], BF, tag="xgT")
                for k, src in enumerate(srcs):
                    eng.dma_start_transpose(
                        xgT[:, :, k, :],
                        src[:, j * 256:(j + 1) * 256])
                return xgT

            def weight_main(is_gate, j, xgT):
                o = OG if is_gate else OU
                wsrc = GW if is_gate else UW
                wod = WOG if is_gate else WOU
                chunks = GCH if is_gate else UCH

                acc = accp.tile([128, B2, o], BF, tag=f"acc{o}")
                for ci, (q0, w) in enumerate(chunks):
                    pw = pB.tile([128, B2, 512], F32, tag="big", name="pw")
                    for b in range(B2):
                        for k in range(K):
                            nc.tensor.matmul(
                                pw[:, b, 0:w],
                                xgT[:, b, k, :],
                                wsrc[:, k * wod + q0: k * wod + q0 + w],
                                start=(k == 0), stop=(k == K - 1))
                    sc = scp.tile([128, B2, 512], BF, tag="sc")
                    e4 = ET[:, j * D:(j + 1) * D].unsqueeze(1).unsqueeze(1) \
                        .broadcast_to([128, B2, w // D, D])
                    nc.vector.tensor_tensor(
                        sc[:, :, 0:w].rearrange("p b (o d) -> p b o d", d=D),
                        pw[:, :, 0:w].rearrange("p b (o d) -> p b o d", d=D),
                        e4, MULT)
                    with nc.allow_low_precision(reason="d-sum, 10 terms"):
                        nc.vector.tensor_reduce(
                            acc[:, :, q0 // D: (q0 + w) // D],
                            sc[:, :, 0:w].rearrange("p b (o d) -> p b o d",
                                                    d=D),
                            AXX, ADD)
                if is_gate:
                    gbj = GB[:, j * OG:(j + 1) * OG].unsqueeze(1) \
                        .broadcast_to([128, B2, OG])
                    nc.gpsimd.tensor_tensor(acc[:], acc[:], gbj, ADD)
                    sig = sgp.tile([128, B2 * OG], BF, tag="sig")
                    nc.scalar.activation(sig[:], acc[:].rearrange(
                        "p b o -> p (b o)"), AFT.Sigmoid)
                    sigT = sgp.tile([128, B2, OG], BF, tag="sigT")
                    nc.scalar.dma_start_transpose(sigT[:], sig[:])
                    pz2 = pD.tile([128, B2 * C], F32, tag="pd", name="pz2")
                    for b in range(B2):
                        nc.tensor.matmul(pz2[:, b * C:(b + 1) * C],
                                         sigT[:, b, :], LW[:],
                                         start=True, stop=True)
                    # z_r written straight into [x|zr] cols [64:128]
                    lbb = LB[:].unsqueeze(1).broadcast_to([128, B2, C])
                    nc.vector.tensor_tensor(
                        cat3(XZ, j)[:, :, C:C2],
                        pz2[:].rearrange("p (b c) -> p b c", b=B2),
                        lbb, ADD)
                else:
                    ubj = UB[:, j * OU:(j + 1) * OU].unsqueeze(1) \
                        .broadcast_to([128, B2, OU])
                    nc.gpsimd.tensor_tensor(acc[:], acc[:], ubj, ADD)
                    nc.scalar.activation(
                        OUT[:, j * (B2 * C):(j + 1) * (B2 * C)],
                        acc[:].rearrange("p b o -> p (b o)"), AFT.Tanh)

            # D1b interleaved with the gate weight phase: PE streams U2
            # matmuls while DVE drains the previous block's d-contraction
            prev = None
            for j in range(MT):
                hop_j(j, lambda m: Y1[:, m * 256:(m + 1) * 256],
                      full_evict(U2, j))
                nc.gpsimd.tensor_copy(cat3(UZ, j)[:, :, 0:C],
                                      cat3(U2, j)[:, :, 0:C])
                cur = weight_prep(True, j)
                if prev is not None:
                    weight_main(True, j - 1, prev)
                prev = cur
            weight_main(True, MT - 1, prev)

            def zr_rhs(m):
                return cat3(XZ, m)[:, :, C:C2]

            def y1z_rhs(m):
                return cat3(YZ, m)[:, :, C:C2]

            for j in range(MT):
                hop_j(j, zr_rhs, z_evict(YZ, j))
            prev = None
            for j in range(MT):
                hop_j(j, y1z_rhs, z_evict(UZ, j))
                cur = weight_prep(False, j)
                if prev is not None:
                    weight_main(False, j - 1, prev)
                    if j % 2 == 0 and j >= 2:
                        lo = (j - 2) * B2 * C
                        nc.sync.dma_start(out_d[:, lo:lo + 2 * B2 * C],
                                          OUT[:, lo:lo + 2 * B2 * C])
                prev = cur
            weight_main(False, MT - 1, prev)
            lo = (MT - 2) * B2 * C
            nc.sync.dma_start(out_d[:, lo:], OUT[:, lo:])

    nc.compile()
    return nc


def kernel(x, state, node_embeddings, gate_wpool, gate_bpool,
           upd_wpool, upd_bpool, lin_w, lin_b):
    x = np.asarray(x, np.float32)
    state = np.asarray(state, np.float32)
    E = np.asarray(node_embeddings, np.float32)
    gw = np.asarray(gate_wpool, np.float32)
    gb = np.asarray(gate_bpool, np.float32)
    uw = np.asarray(upd_wpool, np.float32)
    ub = np.asarray(upd_bpool, np.float32)
    lw = np.asarray(lin_w, np.float32)
    lb = np.asarray(lin_b, np.float32)
    bf = ml_dtypes.bfloat16

    if "nc" not in _CACHE:
        _CACHE["nc"] = _build()
    nc = _CACHE["nc"]

    def fold_cheb(w):
        # w: [D, K, Ci, O] -> w0-w2, w1, 2*w2 then [c2, k, o, d] tiling
        wm = np.stack([w[:, 0] - w[:, 2], w[:, 1], 2.0 * w[:, 2]], axis=1)
        return np.ascontiguousarray(
            wm.transpose(2, 1, 3, 0).reshape(C2, -1)).astype(bf)

    gwr = fold_cheb(gw)                           # [128, 3840]
    uwr = fold_cheb(uw)                           # [128, 1920]
    gbf = (E @ gb).reshape(MT, 128, OG).transpose(1, 0, 2) \
        .reshape(128, MT * OG).astype(bf)
    ubf = (E @ ub).reshape(MT, 128, OU).transpose(1, 0, 2) \
        .reshape(128, MT * OU).astype(bf)
    etr = np.ascontiguousarray(E.T)               # [10, 2048] f32
    ett = E.reshape(MT, 128, D).transpose(1, 0, 2).reshape(128, MT * D)
    ett = np.ascontiguousarray(ett)
    lwT = np.ascontiguousarray(lw.T).astype(bf)   # [128, 64]
    lbt = np.ascontiguousarray(np.tile(lb[None, :], (128, 1)))

    xcat = np.concatenate([x, state], axis=-1)    # [16, 2048, 128] f32

    in_maps = []
    for r in range(NCORES):
        xcr = xcat[2 * r:2 * r + 2].reshape(B2, MT, 128, C2) \
            .transpose(2, 1, 0, 3).reshape(128, MT * B2 * C2)
        in_maps.append({
            "xc": np.ascontiguousarray(xcr).astype(bf),
            "ew": etr, "et": ett,
            "gw": gwr, "uw": uwr,
            "gb": gbf, "ub": ubf,
            "lw": lwT, "lb": lbt,
        })
    global _LAST_IN_MAPS
    _LAST_IN_MAPS = in_maps
    res = run_bass_kernel_spmd(nc, in_maps, core_ids=list(range(NCORES)))
    outs = []
    for r in range(NCORES):
        o = res.results[r]["out"]                  # [128, 16*2*64] (j,b,c)
        o = o.reshape(128, MT, B2, C).transpose(2, 1, 0, 3) \
            .reshape(B2, N, C)
        outs.append(o)
    return np.concatenate(outs, axis=0).astype(np.float32)


# revision 14
# speedup vs baseline: 1.0470x; 1.0470x over previous
"""AGCRNCell distributed Bass kernel for 8 TRN2 NeuronCores.

Batch-parallel: B=16 -> 2 batches/core, zero collectives.  Each core:
  A = exp(relu(E @ E^T))      (symmetric -> A^T = A, no transposes;
                               softmax normalization deferred: S@v =
                               rinv * (A@v), rinv applied at eviction)
  diffusion hops as dense bf16 matmuls over 128x128 tiles of A,
  Chebyshev term folded into the weight pools host-side:
      sum_k xg_k w_k = xg0 (w0-w2) + y1 w1 + u2 (2 w2),
      y1 = rinv*(A@x), u2 = rinv*(A@y1)
  per-node adaptive weights factored through the embedding dim D=10:
      out[n,o] = sum_d E[n,d] * (xg[n,:] @ wpool[d,:,(o)])
  with weight pools laid out (o,d)-interleaved so the d-contraction is
  one DVE tensor_tensor(mult, E broadcast) + one tensor_reduce(X) per
  chunk, batched over both local batches.

v2: HAM warmup burst (PE idles ~90us at 1.2GHz otherwise), fused
1024-wide exp eviction via a shared 2-bank PSUM tag, weight phase
batched over b (half the DVE/scalar instructions), bf16 d-sum
accumulate, j-major OUT layout with streamed DMA.
"""

import numpy as np
import ml_dtypes

import concourse.bass as bass
import concourse.mybir as mybir
import concourse.tile as tile
from concourse import bacc
from concourse.bass_utils import run_bass_kernel_spmd
from concourse.masks import make_identity

BF = mybir.dt.bfloat16
F32 = mybir.dt.float32
F32R = mybir.dt.float32r

B, N, C, D, K = 16, 2048, 64, 10, 3
NCORES = 8
B2 = B // NCORES          # 2 batches per core
MT = N // 128             # 16 row tiles
C2 = 2 * C                # 128
OG = 2 * C                # gate output width 128
OU = C                    # update output width 64
WOG = OG * D              # 1280 (o,d)-interleaved gate width
WOU = OU * D              # 640
GCH = [(0, 510), (510, 510), (1020, 260)]   # gate (o,d) chunks, mult of 10
UCH = [(0, 510), (510, 130)]                # update chunks

_CACHE = {}


def _build():
    nc = bacc.Bacc("TRN2", target_bir_lowering=False, debug=False,
                   num_devices=NCORES)

    def inp(name, shape, dt):
        return nc.dram_tensor(name, list(shape), dt, kind="ExternalInput").ap()

    xc_d = inp("xc", (128, MT * B2 * C2), BF)    # [p, (m,b,c2)] xcat tiles
    ew_d = inp("ew", (D, N), F32R)               # E^T
    et_d = inp("et", (128, MT * D), F32)         # [p, (j,d)] E rows
    gw_d = inp("gw", (128, K * WOG), BF)         # [c2, (k,o,d)] gate pool
    uw_d = inp("uw", (128, K * WOU), BF)
    gb_d = inp("gb", (128, MT * OG), BF)         # [p, (j,o)] E@gate_bpool
    ub_d = inp("ub", (128, MT * OU), BF)
    lw_d = inp("lw", (C2, C), BF)                # lin_w^T
    lb_d = inp("lb", (128, C), F32)              # lin_b tiled
    out_d = nc.dram_tensor("out", [128, MT * B2 * C], F32,
                           kind="ExternalOutput").ap()   # [p, (j,b,c)]

    AFT = mybir.ActivationFunctionType
    MULT = mybir.AluOpType.mult
    ADD = mybir.AluOpType.add
    AXX = mybir.AxisListType.X

    with tile.TileContext(nc) as tc:
        with (
            tc.tile_pool(name="const", bufs=1) as const,
            tc.tile_pool(name="sraw", bufs=2) as srp,
            tc.tile_pool(name="stat", bufs=4) as stat,
            tc.tile_pool(name="xgp", bufs=4) as xgp,
            tc.tile_pool(name="scp", bufs=4) as scp,
            tc.tile_pool(name="accp", bufs=4) as accp,
            tc.tile_pool(name="sgp", bufs=4) as sgp,
            # PSUM: "big" 2 banks x3, "pd" 1x2 = 8 banks
            tc.tile_pool(name="pB", bufs=3, space="PSUM") as pB,
            tc.tile_pool(name="pD", bufs=2, space="PSUM") as pD,
        ):
            ident = const.tile([128, 128], BF)
            make_identity(nc, ident[:])

            # ---- HAM warmup: dense matmuls so the PE clock un-throttles
            # (K=4/8 -> 8/8) and stays there while the input DMAs land.
            # Small tensors (EW for EE^T) are DMA'd first so real PE work
            # starts within ~2us.
            wrm = const.tile([128, 512], BF)
            nc.gpsimd.memset(wrm[:], 0.0)

            EW = const.tile([D, N], F32R)
            nc.sync.dma_start(EW[:], ew_d[:])
            ET = const.tile([128, MT * D], F32)
            nc.sync.dma_start(ET[:], et_d[:])
            LW = const.tile([C2, C], BF)
            nc.sync.dma_start(LW[:], lw_d[:])
            LB = const.tile([128, C], F32)
            nc.sync.dma_start(LB[:], lb_d[:])
            GB = const.tile([128, MT * OG], BF)
            nc.sync.dma_start(GB[:], gb_d[:])
            UB = const.tile([128, MT * OU], BF)
            nc.sync.dma_start(UB[:], ub_d[:])
            XC = const.tile([128, MT * B2 * C2], BF)
            nc.sync.dma_start(XC[:], xc_d[:])
            GW = const.tile([128, K * WOG], BF)
            nc.sync.dma_start(GW[:], gw_d[:])
            UW = const.tile([128, K * WOU], BF)
            nc.sync.dma_start(UW[:], uw_d[:])

            for i in range(10):
                pwm = pB.tile([128, 1024], F32, tag="big", name="pwm")
                nc.tensor.matmul(pwm[:, 0:512], ident[:], wrm[:],
                                 start=True, stop=True)

            A = [const.tile([128, N], BF, tag=f"A{j}", name=f"A{j}")
                 for j in range(MT)]
            RINV = const.tile([128, MT], F32)
            OUT = const.tile([128, MT * B2 * C], F32)   # [p, (j,b,c)]

            # persistent diffusion state
            Y1 = const.tile([128, MT * B2 * C2], BF)    # rinv*(A@[x|s])
            U2 = const.tile([128, MT * B2 * C2], BF)    # rinv*(A@Y1)
            # stitched [x|zr], [y1x|y1z], [u2x|u2z] per (j, b): 64+64 cols
            XZ = const.tile([128, MT * B2 * C2], BF)
            YZ = const.tile([128, MT * B2 * C2], BF)
            UZ = const.tile([128, MT * B2 * C2], BF)

            def cat3(t, j):
                return t[:, j * 256:(j + 1) * 256] \
                    .rearrange("p (b c) -> p b c", b=B2)

            # partial accumulators for the split first hop (m 0..7 during
            # phase S, m 8..15 after)
            PRT = const.tile([128, MT * B2 * C2], BF)

            def d1a_passA(j):
                pd = pD.tile([128, B2 * C2], F32, tag="pd")
                for m in range(MT // 2):
                    nc.tensor.matmul(pd[:],
                                     A[m][:, j * 128:(j + 1) * 128],
                                     XC[:, m * 256:(m + 1) * 256],
                                     start=(m == 0), stop=(m == MT // 2 - 1))
                nc.scalar.activation(PRT[:, j * 256:(j + 1) * 256], pd[:],
                                     AFT.Copy, scale=RINV[:, j:j + 1])

            # ---- phase S: A = max(exp(E@E^T), 1), rinv = 1/rowsum ----
            # (exp(relu(x)) == max(exp(x), 1); clamp+rowsum fused on DVE).
            # EE^T lands in 2-bank PSUM tiles so exp evicts 1024 wide.
            # Once the first 8 A-tiles exist, the first half of the A@[x|s]
            # hop runs interleaved to keep the PE busy under S's DVE chain.
            for j in range(MT):
                etmp = srp.tile([128, N], BF, tag="etmp")
                for h in range(2):
                    ps = pB.tile([128, 1024], F32, tag="big", name="ps")
                    for q in range(2):
                        nc.tensor.matmul(ps[:, q * 512:(q + 1) * 512],
                                         EW[:, j * 128:(j + 1) * 128],
                                         EW[:, (2 * h + q) * 512:
                                             (2 * h + q + 1) * 512],
                                         start=True, stop=True)
                    nc.scalar.activation(etmp[:, h * 1024:(h + 1) * 1024],
                                         ps[:], AFT.Exp)
                zs = stat.tile([128, 1], F32, tag="zs")
                nc.vector.tensor_scalar(A[j][:], etmp[:], 1.0, 0.0,
                                        mybir.AluOpType.max,
                                        mybir.AluOpType.add,
                                        accum_out=zs[:])
                nc.vector.reciprocal(RINV[:, j:j + 1], zs[:])
                if j >= MT // 2:
                    d1a_passA(j - MT // 2)
                # keep PE from idling a full HAM window while the
                # scalar/DVE chain drains (idle >3.4us re-throttles
                # the clock to 1.2GHz for the rest of phase S)
                pdw = pD.tile([128, B2 * C2], F32, tag="pd", name="pdw")
                for _ in range(4 if j < MT // 2 else 2):
                    nc.tensor.matmul(pdw[:], ident[:], wrm[:, 0:256],
                                     start=True, stop=True)
            for j in range(MT // 2, MT):
                d1a_passA(j)

            # ---- diffusion hop: dst_j = rinv_j * (A @ rhs) ----
            def hop_j(j, rhs_fn, evict_fn, m0=0):
                pd = pD.tile([128, B2 * C2], F32, tag="pd")
                w = rhs_fn(0).free_size()
                for m in range(m0, MT):
                    nc.tensor.matmul(pd[:, 0:w],
                                     A[m][:, j * 128:(j + 1) * 128],
                                     rhs_fn(m),
                                     start=(m == m0), stop=(m == MT - 1))
                evict_fn(pd[:, 0:w])

            def full_evict(dst, j):
                def ev(pdw):
                    nc.scalar.activation(dst[:, j * 256:(j + 1) * 256], pdw,
                                         AFT.Copy, scale=RINV[:, j:j + 1])
                return ev

            def z_evict(dst, j):
                # write z-halves into cols [64:128] of each 128-col group
                def ev(pdw):
                    nc.scalar.activation(
                        cat3(dst, j)[:, :, C:C2],
                        pdw.rearrange("p (b c) -> p b c", b=B2),
                        AFT.Copy, scale=RINV[:, j:j + 1])
                return ev

            for j in range(MT):
                # second half of the first hop; fused add of the pass-A
                # partial during eviction
                def evA(pdw, j=j):
                    nc.vector.scalar_tensor_tensor(
                        Y1[:, j * 256:(j + 1) * 256], pdw,
                        RINV[:, j:j + 1],
                        PRT[:, j * 256:(j + 1) * 256],
                        MULT, ADD)
                hop_j(j, lambda m: XC[:, m * 256:(m + 1) * 256], evA,
                      m0=MT // 2)
                # prefill x / y1x columns of the stitched tiles (gpsimd:
                # SBUF-to-SBUF, keeps scalar/DVE free)
                nc.gpsimd.tensor_copy(cat3(XZ, j)[:, :, 0:C],
                                      cat3(XC, j)[:, :, 0:C])
                nc.gpsimd.tensor_copy(cat3(YZ, j)[:, :, 0:C],
                                      cat3(Y1, j)[:, :, 0:C])

            # ---- weight application: prep (transposes) + main, software-
            # pipelined one j apart so the PE never heads-of-line blocks on
            # the cross-engine chain.  Both local batches are processed in
            # one batched PSUM tile / DVE op per chunk.
            def weight_prep(is_gate, j):
                # xg transposes on the DMA xbar (idle engine) instead of
                # PE+scalar: one batched 16x128-tiled transpose per source,
                # covering both local batches.
                srcs = (XC, Y1, U2) if is_gate else (XZ, YZ, UZ)
                xgT = xgp.tile([128, B2, K, 128], BF, tag="xgT")
                for k, src in enumerate(srcs):
                    # descriptor generation costs ~1.2us of issuing-engine
                    # time per transpose; spread across sync and scalar
                    eng = nc.scalar if k == 0 else nc.sync
                    eng.dma_start_transpose(
                        xgT[:, :, k, :],
                        src[:, j * 256:(j + 1) * 256])
                return xgT

            def weight_main(is_gate, j, xgT):
                o = OG if is_gate else OU
                wsrc = GW if is_gate else UW
                wod = WOG if is_gate else WOU
                chunks = GCH if is_gate else UCH

                acc = accp.tile([128, B2, o], BF, tag=f"acc{o}")
                for ci, (q0, w) in enumerate(chunks):
                    pw = pB.tile([128, B2, 512], F32, tag="big", name="pw")
                    for b in range(B2):
                        for k in range(K):
                            nc.tensor.matmul(
                                pw[:, b, 0:w],
                                xgT[:, b, k, :],
                                wsrc[:, k * wod + q0: k * wod + q0 + w],
                                start=(k == 0), stop=(k == K - 1))
                    sc = scp.tile([128, B2, 512], BF, tag="sc")
                    e4 = ET[:, j * D:(j + 1) * D].unsqueeze(1).unsqueeze(1) \
                        .broadcast_to([128, B2, w // D, D])
                    nc.vector.tensor_tensor(
                        sc[:, :, 0:w].rearrange("p b (o d) -> p b o d", d=D),
                        pw[:, :, 0:w].rearrange("p b (o d) -> p b o d", d=D),
                        e4, MULT)
                    with nc.allow_low_precision(reason="d-sum, 10 terms"):
                        nc.vector.tensor_reduce(
                            acc[:, :, q0 // D: (q0 + w) // D],
                            sc[:, :, 0:w].rearrange("p b (o d) -> p b o d",
                                                    d=D),
                            AXX, ADD)
                if is_gate:
                    gbj = GB[:, j * OG:(j + 1) * OG].unsqueeze(1) \
                        .broadcast_to([128, B2, OG])
                    nc.gpsimd.tensor_tensor(acc[:], acc[:], gbj, ADD)
                    sig = sgp.tile([128, B2 * OG], BF, tag="sig")
                    nc.scalar.activation(sig[:], acc[:].rearrange(
                        "p b o -> p (b o)"), AFT.Sigmoid)
                    sigT = sgp.tile([128, B2, OG], BF, tag="sigT")
                    nc.scalar.dma_start_transpose(sigT[:], sig[:])
                    pz2 = pD.tile([128, B2 * C], F32, tag="pd", name="pz2")
                    for b in range(B2):
                        nc.tensor.matmul(pz2[:, b * C:(b + 1) * C],
                                         sigT[:, b, :], LW[:],
                                         start=True, stop=True)
                    # z_r written straight into [x|zr] cols [64:128]
                    lbb = LB[:].unsqueeze(1).broadcast_to([128, B2, C])
                    nc.vector.tensor_tensor(
                        cat3(XZ, j)[:, :, C:C2],
                        pz2[:].rearrange("p (b c) -> p b c", b=B2),
                        lbb, ADD)
                else:
                    ubj = UB[:, j * OU:(j + 1) * OU].unsqueeze(1) \
                        .broadcast_to([128, B2, OU])
                    nc.gpsimd.tensor_tensor(acc[:], acc[:], ubj, ADD)
                    nc.scalar.activation(
                        OUT[:, j * (B2 * C):(j + 1) * (B2 * C)],
                        acc[:].rearrange("p b o -> p (b o)"), AFT.Tanh)

            # D1b interleaved with the gate weight phase: PE streams U2
            # matmuls while DVE drains the previous block's d-contraction
            prev = None
            for j in range(MT):
                hop_j(j, lambda m: Y1[:, m * 256:(m + 1) * 256],
                      full_evict(U2, j))
                nc.gpsimd.tensor_copy(cat3(UZ, j)[:, :, 0:C],
                                      cat3(U2, j)[:, :, 0:C])
                cur = weight_prep(True, j)
                if prev is not None:
                    weight_main(True, j - 1, prev)
                prev = cur
            weight_main(True, MT - 1, prev)

            def zr_rhs(m):
                return cat3(XZ, m)[:, :, C:C2]

            def y1z_rhs(m):
                return cat3(YZ, m)[:, :, C:C2]

            for j in range(MT):
                hop_j(j, zr_rhs, z_evict(YZ, j))
            prev = None
            for j in range(MT):
                hop_j(j, y1z_rhs, z_evict(UZ, j))
                cur = weight_prep(False, j)
                if prev is not None:
                    weight_main(False, j - 1, prev)
                    if j % 2 == 0 and j >= 2:
                        lo = (j - 2) * B2 * C
                        nc.sync.dma_start(out_d[:, lo:lo + 2 * B2 * C],
                                          OUT[:, lo:lo + 2 * B2 * C])
                prev = cur
            weight_main(False, MT - 1, prev)
            lo = (MT - 2) * B2 * C
            nc.sync.dma_start(out_d[:, lo:], OUT[:, lo:])

    nc.compile()
    return nc


def kernel(x, state, node_embeddings, gate_wpool, gate_bpool,
           upd_wpool, upd_bpool, lin_w, lin_b):
    x = np.asarray(x, np.float32)
    state = np.asarray(state, np.float32)
    E = np.asarray(node_embeddings, np.float32)
    gw = np.asarray(gate_wpool, np.float32)
    gb = np.asarray(gate_bpool, np.float32)
    uw = np.asarray(upd_wpool, np.float32)
    ub = np.asarray(upd_bpool, np.float32)
    lw = np.asarray(lin_w, np.float32)
    lb = np.asarray(lin_b, np.float32)
    bf = ml_dtypes.bfloat16

    if "nc" not in _CACHE:
        _CACHE["nc"] = _build()
    nc = _CACHE["nc"]

    def fold_cheb(w):
        # w: [D, K, Ci, O] -> w0-w2, w1, 2*w2 then [c2, k, o, d] tiling
        wm = np.stack([w[:, 0] - w[:, 2], w[:, 1], 2.0 * w[:, 2]], axis=1)
        return np.ascontiguousarray(
            wm.transpose(2, 1, 3, 0).reshape(C2, -1)).astype(bf)

    gwr = fold_cheb(gw)                           # [128, 3840]
    uwr = fold_cheb(uw)                           # [128, 1920]
    gbf = (E @ gb).reshape(MT, 128, OG).transpose(1, 0, 2) \
        .reshape(128, MT * OG).astype(bf)
    ubf = (E @ ub).reshape(MT, 128, OU).transpose(1, 0, 2) \
        .reshape(128, MT * OU).astype(bf)
    etr = np.ascontiguousarray(E.T)               # [10, 2048] f32
    ett = E.reshape(MT, 128, D).transpose(1, 0, 2).reshape(128, MT * D)
    ett = np.ascontiguousarray(ett)
    lwT = np.ascontiguousarray(lw.T).astype(bf)   # [128, 64]
    lbt = np.ascontiguousarray(np.tile(lb[None, :], (128, 1)))

    xcat = np.concatenate([x, state], axis=-1)    # [16, 2048, 128] f32

    in_maps = []
    for r in range(NCORES):
        xcr = xcat[2 * r:2 * r + 2].reshape(B2, MT, 128, C2) \
            .transpose(2, 1, 0, 3).reshape(128, MT * B2 * C2)
        in_maps.append({
            "xc": np.ascontiguousarray(xcr).astype(bf),
            "ew": etr, "et": ett,
            "gw": gwr, "uw": uwr,
            "gb": gbf, "ub": ubf,
            "lw": lwT, "lb": lbt,
        })
    global _LAST_IN_MAPS
    _LAST_IN_MAPS = in_maps
    res = run_bass_kernel_spmd(nc, in_maps, core_ids=list(range(NCORES)))
    outs = []
    for r in range(NCORES):
        o = res.results[r]["out"]                  # [128, 16*2*64] (j,b,c)
        o = o.reshape(128, MT, B2, C).transpose(2, 1, 0, 3) \
            .reshape(B2, N, C)
        outs.append(o)
    return np.concatenate(outs, axis=0).astype(np.float32)


# revision 18
# speedup vs baseline: 1.1990x; 1.1451x over previous
"""AGCRNCell distributed Bass kernel for 8 TRN2 NeuronCores.

Batch-parallel: B=16 -> 2 batches/core, zero collectives.  Each core:
  A = exp(relu(E @ E^T))      (symmetric -> A^T = A, no transposes;
                               softmax normalization deferred: S@v =
                               rinv * (A@v), rinv applied at eviction)
  diffusion hops as dense bf16 matmuls over 128x128 tiles of A,
  Chebyshev term folded into the weight pools host-side:
      sum_k xg_k w_k = xg0 (w0-w2) + y1 w1 + u2 (2 w2),
      y1 = rinv*(A@x), u2 = rinv*(A@y1)
  per-node adaptive weights factored through the embedding dim D=10:
      out[n,o] = sum_d E[n,d] * (xg[n,:] @ wpool[d,:,(o)])
  with weight pools laid out (o,d)-interleaved so the d-contraction is
  one DVE tensor_tensor(mult, E broadcast) + one tensor_reduce(X) per
  chunk, batched over both local batches.

v2: HAM warmup burst (PE idles ~90us at 1.2GHz otherwise), fused
1024-wide exp eviction via a shared 2-bank PSUM tag, weight phase
batched over b (half the DVE/scalar instructions), bf16 d-sum
accumulate, j-major OUT layout with streamed DMA.
"""

import numpy as np
import ml_dtypes

import concourse.bass as bass
import concourse.mybir as mybir
import concourse.tile as tile
from concourse import bacc
from concourse.bass_utils import run_bass_kernel_spmd
from concourse.masks import make_identity

BF = mybir.dt.bfloat16
F32 = mybir.dt.float32
F32R = mybir.dt.float32r

B, N, C, D, K = 16, 2048, 64, 10, 3
NCORES = 8
B2 = B // NCORES          # 2 batches per core
MT = N // 128             # 16 row tiles
C2 = 2 * C                # 128
OG = 2 * C                # gate output width 128
OU = C                    # update output width 64
WOG = OG * D              # 1280 (o,d)-interleaved gate width
WOU = OU * D              # 640
GCH = [(0, 510), (510, 510), (1020, 260)]   # gate (o,d) chunks, mult of 10
UCH = [(0, 510), (510, 130)]                # update chunks

_CACHE = {}


def _build():
    nc = bacc.Bacc("TRN2", target_bir_lowering=False, debug=False,
                   num_devices=NCORES)

    def inp(name, shape, dt):
        return nc.dram_tensor(name, list(shape), dt, kind="ExternalInput").ap()

    xc_d = inp("xc", (128, MT * B2 * C2), BF)    # [p, (m,b,c2)] xcat tiles
    ew_d = inp("ew", (D, N), F32R)               # E^T
    et_d = inp("et", (128, MT * D), F32)         # [p, (j,d)] E rows
    gw_d = inp("gw", (128, K * WOG), BF)         # [c2, (k,o,d)] gate pool
    uw_d = inp("uw", (128, K * WOU), BF)
    gb_d = inp("gb", (128, MT * OG), BF)         # [p, (j,o)] E@gate_bpool
    ub_d = inp("ub", (128, MT * OU), BF)
    lw_d = inp("lw", (C2, C), BF)                # lin_w^T
    lb_d = inp("lb", (128, C), F32)              # lin_b tiled
    out_d = nc.dram_tensor("out", [128, MT * B2 * C], F32,
                           kind="ExternalOutput").ap()   # [p, (j,b,c)]

    AFT = mybir.ActivationFunctionType
    MULT = mybir.AluOpType.mult
    ADD = mybir.AluOpType.add
    AXX = mybir.AxisListType.X

    with tile.TileContext(nc) as tc:
        with (
            tc.tile_pool(name="const", bufs=1) as const,
            tc.tile_pool(name="sraw", bufs=4) as srp,
            tc.tile_pool(name="stat", bufs=4) as stat,
            tc.tile_pool(name="xgp", bufs=4) as xgp,
            tc.tile_pool(name="scp", bufs=4) as scp,
            tc.tile_pool(name="accp", bufs=4) as accp,
            tc.tile_pool(name="sgp", bufs=4) as sgp,
            # PSUM: "big" 2 banks x2, "pd" 1x2, "pt" 1x2 = 8 banks
            tc.tile_pool(name="pB", bufs=2, space="PSUM") as pB,
            tc.tile_pool(name="pD", bufs=2, space="PSUM") as pD,
            tc.tile_pool(name="pT", bufs=2, space="PSUM") as pT,
        ):
            ident = const.tile([128, 128], BF)
            make_identity(nc, ident[:])

            # ---- HAM warmup: dense matmuls so the PE clock un-throttles
            # (K=4/8 -> 8/8) and stays there while the input DMAs land.
            # Small tensors (EW for EE^T) are DMA'd first so real PE work
            # starts within ~2us.
            wrm = const.tile([128, 512], BF)
            nc.gpsimd.memset(wrm[:], 0.0)

            EW = const.tile([D, N], F32R)
            nc.sync.dma_start(EW[:], ew_d[:])
            ET = const.tile([128, MT * D], F32)
            nc.sync.dma_start(ET[:], et_d[:])
            LW = const.tile([C2, C], BF)
            nc.sync.dma_start(LW[:], lw_d[:])
            LB = const.tile([128, C], F32)
            nc.sync.dma_start(LB[:], lb_d[:])
            GB = const.tile([128, MT * OG], BF)
            nc.sync.dma_start(GB[:], gb_d[:])
            UB = const.tile([128, MT * OU], BF)
            nc.sync.dma_start(UB[:], ub_d[:])
            XC = const.tile([128, MT * B2 * C2], BF)
            nc.sync.dma_start(XC[:], xc_d[:])
            GW = const.tile([128, K * WOG], BF)
            nc.sync.dma_start(GW[:], gw_d[:])
            UW = const.tile([128, K * WOU], BF)
            nc.sync.dma_start(UW[:], uw_d[:])

            for i in range(10):
                pwm = pB.tile([128, 1024], F32, tag="big", name="pwm")
                nc.tensor.matmul(pwm[:, 0:512], ident[:], wrm[:],
                                 start=True, stop=True)

            A = [const.tile([128, N], BF, tag=f"A{j}", name=f"A{j}")
                 for j in range(MT)]
            RINV = const.tile([128, MT], F32)
            OUT = const.tile([128, MT * B2 * C], F32)   # [p, (j,b,c)]

            # persistent diffusion state
            Y1 = const.tile([128, MT * B2 * C2], BF)    # rinv*(A@[x|s])
            U2 = const.tile([128, MT * B2 * C2], BF)    # rinv*(A@Y1)
            # stitched [x|zr], [y1x|y1z], [u2x|u2z] per (j, b): 64+64 cols
            XZ = const.tile([128, MT * B2 * C2], BF)
            YZ = const.tile([128, MT * B2 * C2], BF)
            UZ = const.tile([128, MT * B2 * C2], BF)

            def cat3(t, j):
                return t[:, j * 256:(j + 1) * 256] \
                    .rearrange("p (b c) -> p b c", b=B2)

            # partial accumulators for the split first hop (m 0..7 during
            # phase S, m 8..15 after)
            PRT = const.tile([128, MT * B2 * C2], BF)

            def d1a_passA(j):
                pd = pD.tile([128, B2 * C2], F32, tag="pd")
                for m in range(MT // 2):
                    nc.tensor.matmul(pd[:],
                                     A[m][:, j * 128:(j + 1) * 128],
                                     XC[:, m * 256:(m + 1) * 256],
                                     start=(m == 0), stop=(m == MT // 2 - 1))
                nc.scalar.activation(PRT[:, j * 256:(j + 1) * 256], pd[:],
                                     AFT.Copy, scale=RINV[:, j:j + 1])

            # ---- phase S: A = max(exp(E@E^T), 1), rinv = 1/rowsum ----
            # (exp(relu(x)) == max(exp(x), 1); clamp+rowsum fused on DVE).
            # EE^T lands in 2-bank PSUM tiles so exp evicts 1024 wide.
            # Once the first 8 A-tiles exist, the first half of the A@[x|s]
            # hop runs interleaved to keep the PE busy under S's DVE chain.
            for j in range(MT):
                etmp = srp.tile([128, N], BF, tag="etmp")
                for h in range(2):
                    ps = pB.tile([128, 1024], F32, tag="big", name="ps")
                    for q in range(2):
                        nc.tensor.matmul(ps[:, q * 512:(q + 1) * 512],
                                         EW[:, j * 128:(j + 1) * 128],
                                         EW[:, (2 * h + q) * 512:
                                             (2 * h + q + 1) * 512],
                                         start=True, stop=True)
                    nc.scalar.activation(etmp[:, h * 1024:(h + 1) * 1024],
                                         ps[:], AFT.Exp)
                zs = stat.tile([128, 1], F32, tag="zs")
                nc.vector.tensor_scalar(A[j][:], etmp[:], 1.0, 0.0,
                                        mybir.AluOpType.max,
                                        mybir.AluOpType.add,
                                        accum_out=zs[:])
                nc.vector.reciprocal(RINV[:, j:j + 1], zs[:])
                if j >= MT // 2:
                    d1a_passA(j - MT // 2)
                # keep PE from idling a full HAM window while the
                # scalar/DVE chain drains (idle >3.4us re-throttles
                # the clock to 1.2GHz for the rest of phase S)
                pdw = pD.tile([128, B2 * C2], F32, tag="pd", name="pdw")
                for _ in range(4 if j < MT // 2 else 2):
                    nc.tensor.matmul(pdw[:], ident[:], wrm[:, 0:256],
                                     start=True, stop=True)
            for j in range(MT // 2, MT):
                d1a_passA(j)

            # ---- diffusion hop: dst_j = rinv_j * (A @ rhs) ----
            def hop_j(j, rhs_fn, evict_fn, m0=0):
                pd = pD.tile([128, B2 * C2], F32, tag="pd")
                w = rhs_fn(0).free_size()
                for m in range(m0, MT):
                    nc.tensor.matmul(pd[:, 0:w],
                                     A[m][:, j * 128:(j + 1) * 128],
                                     rhs_fn(m),
                                     start=(m == m0), stop=(m == MT - 1))
                evict_fn(pd[:, 0:w])

            def full_evict(dst, j):
                def ev(pdw):
                    nc.scalar.activation(dst[:, j * 256:(j + 1) * 256], pdw,
                                         AFT.Copy, scale=RINV[:, j:j + 1])
                return ev

            def z_evict(dst, j):
                # write z-halves into cols [64:128] of each 128-col group
                def ev(pdw):
                    nc.scalar.activation(
                        cat3(dst, j)[:, :, C:C2],
                        pdw.rearrange("p (b c) -> p b c", b=B2),
                        AFT.Copy, scale=RINV[:, j:j + 1])
                return ev

            for j in range(MT):
                # second half of the first hop; fused add of the pass-A
                # partial during eviction
                def evA(pdw, j=j):
                    nc.vector.scalar_tensor_tensor(
                        Y1[:, j * 256:(j + 1) * 256], pdw,
                        RINV[:, j:j + 1],
                        PRT[:, j * 256:(j + 1) * 256],
                        MULT, ADD)
                hop_j(j, lambda m: XC[:, m * 256:(m + 1) * 256], evA,
                      m0=MT // 2)
                # prefill x / y1x columns of the stitched tiles (gpsimd:
                # SBUF-to-SBUF, keeps scalar/DVE free)
                nc.gpsimd.tensor_copy(cat3(XZ, j)[:, :, 0:C],
                                      cat3(XC, j)[:, :, 0:C])
                nc.gpsimd.tensor_copy(cat3(YZ, j)[:, :, 0:C],
                                      cat3(Y1, j)[:, :, 0:C])

            # ---- weight application: prep (transposes) + main, software-
            # pipelined one j apart so the PE never heads-of-line blocks on
            # the cross-engine chain.  Both local batches are processed in
            # one batched PSUM tile / DVE op per chunk.
            def weight_prep(is_gate, j):
                srcs = (XC, Y1, U2) if is_gate else (XZ, YZ, UZ)
                pt = pT.tile([128, B2 * K * 128], BF, tag="pt")
                for b in range(B2):
                    for k, src in enumerate(srcs):
                        nc.tensor.transpose(
                            pt[:, (b * K + k) * 128:(b * K + k + 1) * 128],
                            src[:, j * 256 + b * 128: j * 256 + (b + 1) * 128],
                            ident[:])
                xgT = xgp.tile([128, B2, K, 128], BF, tag="xgT")
                nc.scalar.activation(xgT[:].rearrange("p b k c -> p (b k c)"),
                                     pt[:], AFT.Copy)
                return xgT

            def weight_main(is_gate, j, xgT):
                o = OG if is_gate else OU
                wsrc = GW if is_gate else UW
                wod = WOG if is_gate else WOU
                chunks = GCH if is_gate else UCH

                acc = accp.tile([128, B2, o], BF, tag=f"acc{o}")
                for ci, (q0, w) in enumerate(chunks):
                    pw = pB.tile([128, B2, 512], F32, tag="big", name="pw")
                    for b in range(B2):
                        for k in range(K):
                            nc.tensor.matmul(
                                pw[:, b, 0:w],
                                xgT[:, b, k, :],
                                wsrc[:, k * wod + q0: k * wod + q0 + w],
                                start=(k == 0), stop=(k == K - 1))
                    sc = scp.tile([128, B2, 512], BF, tag="sc")
                    e4 = ET[:, j * D:(j + 1) * D].unsqueeze(1).unsqueeze(1) \
                        .broadcast_to([128, B2, w // D, D])
                    nc.vector.tensor_tensor(
                        sc[:, :, 0:w].rearrange("p b (o d) -> p b o d", d=D),
                        pw[:, :, 0:w].rearrange("p b (o d) -> p b o d", d=D),
                        e4, MULT)
                    with nc.allow_low_precision(reason="d-sum, 10 terms"):
                        nc.vector.tensor_reduce(
                            acc[:, :, q0 // D: (q0 + w) // D],
                            sc[:, :, 0:w].rearrange("p b (o d) -> p b o d",
                                                    d=D),
                            AXX, ADD)
                if is_gate:
                    gbj = GB[:, j * OG:(j + 1) * OG].unsqueeze(1) \
                        .broadcast_to([128, B2, OG])
                    nc.gpsimd.tensor_tensor(acc[:], acc[:], gbj, ADD)
                    sig = sgp.tile([128, B2 * OG], BF, tag="sig")
                    nc.scalar.activation(sig[:], acc[:].rearrange(
                        "p b o -> p (b o)"), AFT.Sigmoid)
                    pts = pT.tile([128, B2 * K * 128], BF, tag="pt")
                    for b in range(B2):
                        nc.tensor.transpose(
                            pts[:, b * 128:(b + 1) * 128],
                            sig[:, b * OG:(b + 1) * OG], ident[:])
                    sigT = sgp.tile([128, B2, OG], BF, tag="sigT")
                    nc.scalar.activation(
                        sigT[:].rearrange("p b o -> p (b o)"),
                        pts[:, 0:B2 * OG], AFT.Copy)
                    pz2 = pD.tile([128, B2 * C], F32, tag="pd", name="pz2")
                    for b in range(B2):
                        nc.tensor.matmul(pz2[:, b * C:(b + 1) * C],
                                         sigT[:, b, :], LW[:],
                                         start=True, stop=True)
                    # z_r written straight into [x|zr] cols [64:128]
                    lbb = LB[:].unsqueeze(1).broadcast_to([128, B2, C])
                    nc.vector.tensor_tensor(
                        cat3(XZ, j)[:, :, C:C2],
                        pz2[:].rearrange("p (b c) -> p b c", b=B2),
                        lbb, ADD)
                else:
                    ubj = UB[:, j * OU:(j + 1) * OU].unsqueeze(1) \
                        .broadcast_to([128, B2, OU])
                    nc.gpsimd.tensor_tensor(acc[:], acc[:], ubj, ADD)
                    nc.scalar.activation(
                        OUT[:, j * (B2 * C):(j + 1) * (B2 * C)],
                        acc[:].rearrange("p b o -> p (b o)"), AFT.Tanh)

            # D1b interleaved with the gate weight phase: PE streams U2
            # matmuls while DVE drains the previous block's d-contraction
            prev = None
            for j in range(MT):
                hop_j(j, lambda m: Y1[:, m * 256:(m + 1) * 256],
                      full_evict(U2, j))
                nc.gpsimd.tensor_copy(cat3(UZ, j)[:, :, 0:C],
                                      cat3(U2, j)[:, :, 0:C])
                cur = weight_prep(True, j)
                if prev is not None:
                    weight_main(True, j - 1, prev)
                prev = cur
            weight_main(True, MT - 1, prev)

            def zr_rhs(m):
                return cat3(XZ, m)[:, :, C:C2]

            def y1z_rhs(m):
                return cat3(YZ, m)[:, :, C:C2]

            for j in range(MT):
                hop_j(j, zr_rhs, z_evict(YZ, j))
            prev = None
            for j in range(MT):
                hop_j(j, y1z_rhs, z_evict(UZ, j))
                cur = weight_prep(False, j)
                if prev is not None:
                    weight_main(False, j - 1, prev)
                    if j % 2 == 0 and j >= 2:
                        lo = (j - 2) * B2 * C
                        nc.sync.dma_start(out_d[:, lo:lo + 2 * B2 * C],
                                          OUT[:, lo:lo + 2 * B2 * C])
                prev = cur
            weight_main(False, MT - 1, prev)
            lo = (MT - 2) * B2 * C
            nc.sync.dma_start(out_d[:, lo:], OUT[:, lo:])

    nc.compile()
    return nc


def kernel(x, state, node_embeddings, gate_wpool, gate_bpool,
           upd_wpool, upd_bpool, lin_w, lin_b):
    x = np.asarray(x, np.float32)
    state = np.asarray(state, np.float32)
    E = np.asarray(node_embeddings, np.float32)
    gw = np.asarray(gate_wpool, np.float32)
    gb = np.asarray(gate_bpool, np.float32)
    uw = np.asarray(upd_wpool, np.float32)
    ub = np.asarray(upd_bpool, np.float32)
    lw = np.asarray(lin_w, np.float32)
    lb = np.asarray(lin_b, np.float32)
    bf = ml_dtypes.bfloat16

    if "nc" not in _CACHE:
        _CACHE["nc"] = _build()
    nc = _CACHE["nc"]

    def fold_cheb(w):
        # w: [D, K, Ci, O] -> w0-w2, w1, 2*w2 then [c2, k, o, d] tiling
        wm = np.stack([w[:, 0] - w[:, 2], w[:, 1], 2.0 * w[:, 2]], axis=1)
        return np.ascontiguousarray(
            wm.transpose(2, 1, 3, 0).reshape(C2, -1)).astype(bf)

    gwr = fold_cheb(gw)                           # [128, 3840]
    uwr = fold_cheb(uw)                           # [128, 1920]
    gbf = (E @ gb).reshape(MT, 128, OG).transpose(1, 0, 2) \
        .reshape(128, MT * OG).astype(bf)
    ubf = (E @ ub).reshape(MT, 128, OU).transpose(1, 0, 2) \
        .reshape(128, MT * OU).astype(bf)
    etr = np.ascontiguousarray(E.T)               # [10, 2048] f32
    ett = E.reshape(MT, 128, D).transpose(1, 0, 2).reshape(128, MT * D)
    ett = np.ascontiguousarray(ett)
    lwT = np.ascontiguousarray(lw.T).astype(bf)   # [128, 64]
    lbt = np.ascontiguousarray(np.tile(lb[None, :], (128, 1)))

    xcat = np.concatenate([x, state], axis=-1)    # [16, 2048, 128] f32

    in_maps = []
    for r in range(NCORES):
        xcr = xcat[2 * r:2 * r + 2].reshape(B2, MT, 128, C2) \
            .transpose(2, 1, 0, 3).reshape(128, MT * B2 * C2)
        in_maps.append({
            "xc": np.ascontiguousarray(xcr).astype(bf),
            "ew": etr, "et": ett,
            "gw": gwr, "uw": uwr,
            "gb": gbf, "ub": ubf,
            "lw": lwT, "lb": lbt,
        })
    global _LAST_IN_MAPS
    _LAST_IN_MAPS = in_maps
    res = run_bass_kernel_spmd(nc, in_maps, core_ids=list(range(NCORES)))
    outs = []
    for r in range(NCORES):
        o = res.results[r]["out"]                  # [128, 16*2*64] (j,b,c)
        o = o.reshape(128, MT, B2, C).transpose(2, 1, 0, 3) \
            .reshape(B2, N, C)
        outs.append(o)
    return np.concatenate(outs, axis=0).astype(np.float32)


# revision 21
# speedup vs baseline: 1.2629x; 1.0533x over previous
"""AGCRNCell distributed Bass kernel for 8 TRN2 NeuronCores.

Batch-parallel: B=16 -> 2 batches/core, zero collectives.  Each core:
  A = exp(relu(E @ E^T))      (symmetric -> A^T = A, no transposes;
                               softmax normalization deferred: S@v =
                               rinv * (A@v), rinv applied at eviction)
  diffusion hops as dense bf16 matmuls over 128x128 tiles of A,
  Chebyshev term folded into the weight pools host-side:
      sum_k xg_k w_k = xg0 (w0-w2) + y1 w1 + u2 (2 w2),
      y1 = rinv*(A@x), u2 = rinv*(A@y1)
  per-node adaptive weights factored through the embedding dim D=10:
      out[n,o] = sum_d E[n,d] * (xg[n,:] @ wpool[d,:,(o)])
  with weight pools laid out (o,d)-interleaved so the d-contraction is
  one DVE tensor_tensor(mult, E broadcast) + one tensor_reduce(X) per
  chunk, batched over both local batches.

v2: HAM warmup burst (PE idles ~90us at 1.2GHz otherwise), fused
1024-wide exp eviction via a shared 2-bank PSUM tag, weight phase
batched over b (half the DVE/scalar instructions), bf16 d-sum
accumulate, j-major OUT layout with streamed DMA.
"""

import numpy as np
import ml_dtypes

import concourse.bass as bass
import concourse.mybir as mybir
import concourse.tile as tile
from concourse import bacc
from concourse.bass_utils import run_bass_kernel_spmd
from concourse.masks import make_identity

BF = mybir.dt.bfloat16
F32 = mybir.dt.float32
F32R = mybir.dt.float32r

B, N, C, D, K = 16, 2048, 64, 10, 3
NCORES = 8
B2 = B // NCORES          # 2 batches per core
MT = N // 128             # 16 row tiles
C2 = 2 * C                # 128
OG = 2 * C                # gate output width 128
OU = C                    # update output width 64
WOG = OG * D              # 1280 (o,d)-interleaved gate width
WOU = OU * D              # 640
GCH = [(0, 510), (510, 510), (1020, 260)]   # gate (o,d) chunks, mult of 10
UCH = [(0, 510), (510, 130)]                # update chunks

_CACHE = {}


def _build():
    nc = bacc.Bacc("TRN2", target_bir_lowering=False, debug=False,
                   num_devices=NCORES)

    def inp(name, shape, dt):
        return nc.dram_tensor(name, list(shape), dt, kind="ExternalInput").ap()

    xc_d = inp("xc", (128, MT * B2 * C2), BF)    # [p, (m,b,c2)] xcat tiles
    ew_d = inp("ew", (D, N), F32R)               # E^T
    et_d = inp("et", (128, MT * D), F32)         # [p, (j,d)] E rows
    gw_d = inp("gw", (128, K * WOG), BF)         # [c2, (k,o,d)] gate pool
    uw_d = inp("uw", (128, K * WOU), BF)
    gb_d = inp("gb", (128, MT * OG), BF)         # [p, (j,o)] E@gate_bpool
    ub_d = inp("ub", (128, MT * OU), BF)
    lw_d = inp("lw", (C2, C), BF)                # lin_w^T
    lb_d = inp("lb", (128, C), F32)              # lin_b tiled
    out_d = nc.dram_tensor("out", [128, MT * B2 * C], F32,
                           kind="ExternalOutput").ap()   # [p, (j,b,c)]

    AFT = mybir.ActivationFunctionType
    MULT = mybir.AluOpType.mult
    ADD = mybir.AluOpType.add
    AXX = mybir.AxisListType.X

    with tile.TileContext(nc) as tc:
        with (
            tc.tile_pool(name="const", bufs=1) as const,
            tc.tile_pool(name="sraw", bufs=4) as srp,
            tc.tile_pool(name="stat", bufs=4) as stat,
            tc.tile_pool(name="xgp", bufs=4) as xgp,
            tc.tile_pool(name="scp", bufs=4) as scp,
            tc.tile_pool(name="accp", bufs=4) as accp,
            tc.tile_pool(name="sgp", bufs=4) as sgp,
            # PSUM: "big" 2 banks x2, "pd" 1x2, "pt" 1x2 = 8 banks
            tc.tile_pool(name="pB", bufs=2, space="PSUM") as pB,
            tc.tile_pool(name="pD", bufs=2, space="PSUM") as pD,
            tc.tile_pool(name="pT", bufs=2, space="PSUM") as pT,
        ):
            ident = const.tile([128, 128], BF)
            make_identity(nc, ident[:])

            # ---- HAM warmup: dense matmuls so the PE clock un-throttles
            # (K=4/8 -> 8/8) and stays there while the input DMAs land.
            # Small tensors (EW for EE^T) are DMA'd first so real PE work
            # starts within ~2us.
            wrm = const.tile([128, 512], BF)
            nc.gpsimd.memset(wrm[:], 0.0)

            EW = const.tile([D, N], F32R)
            nc.sync.dma_start(EW[:], ew_d[:])
            ET = const.tile([128, MT * D], F32)
            nc.sync.dma_start(ET[:], et_d[:])
            LW = const.tile([C2, C], BF)
            nc.sync.dma_start(LW[:], lw_d[:])
            LB = const.tile([128, C], F32)
            nc.sync.dma_start(LB[:], lb_d[:])
            GB = const.tile([128, MT * OG], BF)
            nc.sync.dma_start(GB[:], gb_d[:])
            UB = const.tile([128, MT * OU], BF)
            nc.sync.dma_start(UB[:], ub_d[:])
            XC = const.tile([128, MT * B2 * C2], BF)
            nc.sync.dma_start(XC[:], xc_d[:])
            GW = const.tile([128, K * WOG], BF)
            nc.sync.dma_start(GW[:], gw_d[:])
            UW = const.tile([128, K * WOU], BF)
            nc.sync.dma_start(UW[:], uw_d[:])

            for i in range(10):
                pwm = pB.tile([128, 1024], F32, tag="big", name="pwm")
                nc.tensor.matmul(pwm[:, 0:512], ident[:], wrm[:],
                                 start=True, stop=True)

            A = [const.tile([128, N], BF, tag=f"A{j}", name=f"A{j}")
                 for j in range(MT)]
            RINV = const.tile([128, MT], F32)
            OUT = const.tile([128, MT * B2 * C], F32)   # [p, (j,b,c)]

            # persistent diffusion state
            Y1 = const.tile([128, MT * B2 * C2], BF)    # rinv*(A@[x|s])
            U2 = const.tile([128, MT * B2 * C2], BF)    # rinv*(A@Y1)
            # stitched [x|zr], [y1x|y1z], [u2x|u2z] per (j, b): 64+64 cols
            XZ = const.tile([128, MT * B2 * C2], BF)
            YZ = const.tile([128, MT * B2 * C2], BF)
            UZ = const.tile([128, MT * B2 * C2], BF)

            def cat3(t, j):
                return t[:, j * 256:(j + 1) * 256] \
                    .rearrange("p (b c) -> p b c", b=B2)

            # partial accumulators for the split first hop (m 0..7 during
            # phase S, m 8..15 after)
            PRT = const.tile([128, MT * B2 * C2], BF)

            def d1a_passA(j):
                pd = pD.tile([128, B2 * C2], F32, tag="pd")
                for m in range(MT // 2):
                    nc.tensor.matmul(pd[:],
                                     A[m][:, j * 128:(j + 1) * 128],
                                     XC[:, m * 256:(m + 1) * 256],
                                     start=(m == 0), stop=(m == MT // 2 - 1))
                nc.scalar.activation(PRT[:, j * 256:(j + 1) * 256], pd[:],
                                     AFT.Copy, scale=RINV[:, j:j + 1])

            # ---- phase S: A = max(exp(E@E^T), 1), rinv = 1/rowsum ----
            # (exp(relu(x)) == max(exp(x), 1); clamp+rowsum fused on DVE).
            # EE^T lands in 2-bank PSUM tiles so exp evicts 1024 wide.
            # Once the first 8 A-tiles exist, the first half of the A@[x|s]
            # hop runs interleaved to keep the PE busy under S's DVE chain.
            for j in range(MT):
                etmp = srp.tile([128, N], BF, tag="etmp")
                for h in range(2):
                    ps = pB.tile([128, 1024], F32, tag="big", name="ps")
                    for q in range(2):
                        nc.tensor.matmul(ps[:, q * 512:(q + 1) * 512],
                                         EW[:, j * 128:(j + 1) * 128],
                                         EW[:, (2 * h + q) * 512:
                                             (2 * h + q + 1) * 512],
                                         start=True, stop=True)
                    nc.scalar.activation(etmp[:, h * 1024:(h + 1) * 1024],
                                         ps[:], AFT.Exp)
                zs = stat.tile([128, 1], F32, tag="zs")
                nc.vector.tensor_scalar(A[j][:], etmp[:], 1.0, 0.0,
                                        mybir.AluOpType.max,
                                        mybir.AluOpType.add,
                                        accum_out=zs[:])
                nc.vector.reciprocal(RINV[:, j:j + 1], zs[:])
                if j >= MT // 2:
                    d1a_passA(j - MT // 2)
                    # d1a keeps PE ~80% busy, enough to hold the HAM
                    # clock at 8/8; small filler covers the gaps
                    pdw = pD.tile([128, B2 * C2], F32, tag="pd", name="pdw")
                    for _ in range(2):
                        nc.tensor.matmul(pdw[:], ident[:], wrm[:, 0:256],
                                         start=True, stop=True)
                else:
                    # EE^T alone is ~35% PE-busy at this point and the HAM
                    # re-throttles the PE clock to 1.2GHz; ~2us of filler
                    # matmuls per iteration keeps the busy-window high so
                    # the whole first hop runs at 2.4GHz
                    for _ in range(2):
                        ptw = pT.tile([128, 384], F32, tag="pt", name="ptw")
                        for _ in range(6):
                            nc.tensor.matmul(ptw[:], ident[:],
                                             wrm[:, 0:384],
                                             start=True, stop=True)
            for j in range(MT // 2, MT):
                d1a_passA(j)

            # ---- diffusion hop: dst_j = rinv_j * (A @ rhs) ----
            def hop_j(j, rhs_fn, evict_fn, m0=0):
                pd = pD.tile([128, B2 * C2], F32, tag="pd")
                w = rhs_fn(0).free_size()
                for m in range(m0, MT):
                    nc.tensor.matmul(pd[:, 0:w],
                                     A[m][:, j * 128:(j + 1) * 128],
                                     rhs_fn(m),
                                     start=(m == m0), stop=(m == MT - 1))
                evict_fn(pd[:, 0:w])

            def full_evict(dst, j):
                def ev(pdw):
                    nc.scalar.activation(dst[:, j * 256:(j + 1) * 256], pdw,
                                         AFT.Copy, scale=RINV[:, j:j + 1])
                return ev

            def z_evict(dst, j):
                # write z-halves into cols [64:128] of each 128-col group
                def ev(pdw):
                    nc.scalar.activation(
                        cat3(dst, j)[:, :, C:C2],
                        pdw.rearrange("p (b c) -> p b c", b=B2),
                        AFT.Copy, scale=RINV[:, j:j + 1])
                return ev

            for j in range(MT):
                # second half of the first hop; fused add of the pass-A
                # partial during eviction
                def evA(pdw, j=j):
                    nc.vector.scalar_tensor_tensor(
                        Y1[:, j * 256:(j + 1) * 256], pdw,
                        RINV[:, j:j + 1],
                        PRT[:, j * 256:(j + 1) * 256],
                        MULT, ADD)
                hop_j(j, lambda m: XC[:, m * 256:(m + 1) * 256], evA,
                      m0=MT // 2)
                # prefill x / y1x columns of the stitched tiles (gpsimd:
                # SBUF-to-SBUF, keeps scalar/DVE free)
                nc.gpsimd.tensor_copy(cat3(XZ, j)[:, :, 0:C],
                                      cat3(XC, j)[:, :, 0:C])
                nc.gpsimd.tensor_copy(cat3(YZ, j)[:, :, 0:C],
                                      cat3(Y1, j)[:, :, 0:C])

            # ---- weight application: prep (transposes) + main, software-
            # pipelined one j apart so the PE never heads-of-line blocks on
            # the cross-engine chain.  Both local batches are processed in
            # one batched PSUM tile / DVE op per chunk.
            def weight_prep(is_gate, j):
                srcs = (XC, Y1, U2) if is_gate else (XZ, YZ, UZ)
                pt = pT.tile([128, B2 * K * 128], BF, tag="pt")
                for b in range(B2):
                    for k, src in enumerate(srcs):
                        nc.tensor.transpose(
                            pt[:, (b * K + k) * 128:(b * K + k + 1) * 128],
                            src[:, j * 256 + b * 128: j * 256 + (b + 1) * 128],
                            ident[:])
                xgT = xgp.tile([128, B2, K, 128], BF, tag="xgT")
                nc.scalar.activation(xgT[:].rearrange("p b k c -> p (b k c)"),
                                     pt[:], AFT.Copy)
                return xgT

            def weight_main(is_gate, j, xgT):
                o = OG if is_gate else OU
                wsrc = GW if is_gate else UW
                wod = WOG if is_gate else WOU
                chunks = GCH if is_gate else UCH

                acc = accp.tile([128, B2, o], BF, tag=f"acc{o}")
                for ci, (q0, w) in enumerate(chunks):
                    pw = pB.tile([128, B2, 512], F32, tag="big", name="pw")
                    for b in range(B2):
                        for k in range(K):
                            nc.tensor.matmul(
                                pw[:, b, 0:w],
                                xgT[:, b, k, :],
                                wsrc[:, k * wod + q0: k * wod + q0 + w],
                                start=(k == 0), stop=(k == K - 1))
                    sc = scp.tile([128, B2, 512], BF, tag="sc")
                    e4 = ET[:, j * D:(j + 1) * D].unsqueeze(1).unsqueeze(1) \
                        .broadcast_to([128, B2, w // D, D])
                    nc.vector.tensor_tensor(
                        sc[:, :, 0:w].rearrange("p b (o d) -> p b o d", d=D),
                        pw[:, :, 0:w].rearrange("p b (o d) -> p b o d", d=D),
                        e4, MULT)
                    with nc.allow_low_precision(reason="d-sum, 10 terms"):
                        nc.vector.tensor_reduce(
                            acc[:, :, q0 // D: (q0 + w) // D],
                            sc[:, :, 0:w].rearrange("p b (o d) -> p b o d",
                                                    d=D),
                            AXX, ADD)
                if is_gate:
                    gbj = GB[:, j * OG:(j + 1) * OG].unsqueeze(1) \
                        .broadcast_to([128, B2, OG])
                    nc.gpsimd.tensor_tensor(acc[:], acc[:], gbj, ADD)
                    sig = sgp.tile([128, B2 * OG], BF, tag="sig")
                    nc.scalar.activation(sig[:], acc[:].rearrange(
                        "p b o -> p (b o)"), AFT.Sigmoid)
                    pts = pT.tile([128, B2 * K * 128], BF, tag="pt")
                    for b in range(B2):
                        nc.tensor.transpose(
                            pts[:, b * 128:(b + 1) * 128],
                            sig[:, b * OG:(b + 1) * OG], ident[:])
                    sigT = sgp.tile([128, B2, OG], BF, tag="sigT")
                    nc.scalar.activation(
                        sigT[:].rearrange("p b o -> p (b o)"),
                        pts[:, 0:B2 * OG], AFT.Copy)
                    # pt tag, not pd: a pd-tag pz2 makes the first zr-hop
                    # wait (WAR) on the last gate block's DVE drain,
                    # stalling the PE ~5us at the phase transition
                    pz2 = pT.tile([128, 384], F32, tag="pt", name="pz2")
                    pz2 = pz2[:, 0:B2 * C]
                    for b in range(B2):
                        nc.tensor.matmul(pz2[:, b * C:(b + 1) * C],
                                         sigT[:, b, :], LW[:],
                                         start=True, stop=True)
                    # z_r written straight into [x|zr] cols [64:128]
                    lbb = LB[:].unsqueeze(1).broadcast_to([128, B2, C])
                    nc.vector.tensor_tensor(
                        cat3(XZ, j)[:, :, C:C2],
                        pz2[:].rearrange("p (b c) -> p b c", b=B2),
                        lbb, ADD)
                else:
                    ubj = UB[:, j * OU:(j + 1) * OU].unsqueeze(1) \
                        .broadcast_to([128, B2, OU])
                    nc.gpsimd.tensor_tensor(acc[:], acc[:], ubj, ADD)
                    nc.scalar.activation(
                        OUT[:, j * (B2 * C):(j + 1) * (B2 * C)],
                        acc[:].rearrange("p b o -> p (b o)"), AFT.Tanh)

            # D1b interleaved with the gate weight phase: PE streams U2
            # matmuls while DVE drains the previous block's d-contraction
            prev = None
            for j in range(MT):
                hop_j(j, lambda m: Y1[:, m * 256:(m + 1) * 256],
                      full_evict(U2, j))
                nc.gpsimd.tensor_copy(cat3(UZ, j)[:, :, 0:C],
                                      cat3(U2, j)[:, :, 0:C])
                cur = weight_prep(True, j)
                if prev is not None:
                    weight_main(True, j - 1, prev)
                prev = cur
            weight_main(True, MT - 1, prev)

            def zr_rhs(m):
                return cat3(XZ, m)[:, :, C:C2]

            def y1z_rhs(m):
                return cat3(YZ, m)[:, :, C:C2]

            for j in range(MT):
                hop_j(j, zr_rhs, z_evict(YZ, j))
            prev = None
            for j in range(MT):
                hop_j(j, y1z_rhs, z_evict(UZ, j))
                cur = weight_prep(False, j)
                if prev is not None:
                    weight_main(False, j - 1, prev)
                    lo = (j - 1) * B2 * C
                    nc.sync.dma_start(out_d[:, lo:lo + B2 * C],
                                      OUT[:, lo:lo + B2 * C])
                prev = cur
            weight_main(False, MT - 1, prev)
            lo = (MT - 1) * B2 * C
            nc.sync.dma_start(out_d[:, lo:], OUT[:, lo:])

    nc.compile()
    return nc


def kernel(x, state, node_embeddings, gate_wpool, gate_bpool,
           upd_wpool, upd_bpool, lin_w, lin_b):
    x = np.asarray(x, np.float32)
    state = np.asarray(state, np.float32)
    E = np.asarray(node_embeddings, np.float32)
    gw = np.asarray(gate_wpool, np.float32)
    gb = np.asarray(gate_bpool, np.float32)
    uw = np.asarray(upd_wpool, np.float32)
    ub = np.asarray(upd_bpool, np.float32)
    lw = np.asarray(lin_w, np.float32)
    lb = np.asarray(lin_b, np.float32)
    bf = ml_dtypes.bfloat16

    if "nc" not in _CACHE:
        _CACHE["nc"] = _build()
    nc = _CACHE["nc"]

    def fold_cheb(w):
        # w: [D, K, Ci, O] -> w0-w2, w1, 2*w2 then [c2, k, o, d] tiling
        wm = np.stack([w[:, 0] - w[:, 2], w[:, 1], 2.0 * w[:, 2]], axis=1)
        return np.ascontiguousarray(
            wm.transpose(2, 1, 3, 0).reshape(C2, -1)).astype(bf)

    gwr = fold_cheb(gw)                           # [128, 3840]
    uwr = fold_cheb(uw)                           # [128, 1920]
    gbf = (E @ gb).reshape(MT, 128, OG).transpose(1, 0, 2) \
        .reshape(128, MT * OG).astype(bf)
    ubf = (E @ ub).reshape(MT, 128, OU).transpose(1, 0, 2) \
        .reshape(128, MT * OU).astype(bf)
    etr = np.ascontiguousarray(E.T)               # [10, 2048] f32
    ett = E.reshape(MT, 128, D).transpose(1, 0, 2).reshape(128, MT * D)
    ett = np.ascontiguousarray(ett)
    lwT = np.ascontiguousarray(lw.T).astype(bf)   # [128, 64]
    lbt = np.ascontiguousarray(np.tile(lb[None, :], (128, 1)))

    xcat = np.concatenate([x, state], axis=-1)    # [16, 2048, 128] f32

    in_maps = []
    for r in range(NCORES):
        xcr = xcat[2 * r:2 * r + 2].reshape(B2, MT, 128, C2) \
            .transpose(2, 1, 0, 3).reshape(128, MT * B2 * C2)
        in_maps.append({
            "xc": np.ascontiguousarray(xcr).astype(bf),
            "ew": etr, "et": ett,
            "gw": gwr, "uw": uwr,
            "gb": gbf, "ub": ubf,
            "lw": lwT, "lb": lbt,
        })
    global _LAST_IN_MAPS
    _LAST_IN_MAPS = in_maps
    res = run_bass_kernel_spmd(nc, in_maps, core_ids=list(range(NCORES)))
    outs = []
    for r in range(NCORES):
        o = res.results[r]["out"]                  # [128, 16*2*64] (j,b,c)
        o = o.reshape(128, MT, B2, C).transpose(2, 1, 0, 3) \
            .reshape(B2, N, C)
        outs.append(o)
    return np.concatenate(outs, axis=0).astype(np.float32)


# revision 24
# speedup vs baseline: 1.2873x; 1.0193x over previous
"""AGCRNCell distributed Bass kernel for 8 TRN2 NeuronCores.

Batch-parallel: B=16 -> 2 batches/core, zero collectives.  Each core:
  A = exp(relu(E @ E^T))      (symmetric -> A^T = A, no transposes;
                               softmax normalization deferred: S@v =
                               rinv * (A@v), rinv applied at eviction)
  diffusion hops as dense bf16 matmuls over 128x128 tiles of A,
  Chebyshev term folded into the weight pools host-side:
      sum_k xg_k w_k = xg0 (w0-w2) + y1 w1 + u2 (2 w2),
      y1 = rinv*(A@x), u2 = rinv*(A@y1)
  per-node adaptive weights factored through the embedding dim D=10:
      out[n,o] = sum_d E[n,d] * (xg[n,:] @ wpool[d,:,(o)])
  with weight pools laid out (o,d)-interleaved so the d-contraction is
  one DVE tensor_tensor(mult, E broadcast) + one tensor_reduce(X) per
  chunk, batched over both local batches.

v2: HAM warmup burst (PE idles ~90us at 1.2GHz otherwise), fused
1024-wide exp eviction via a shared 2-bank PSUM tag, weight phase
batched over b (half the DVE/scalar instructions), bf16 d-sum
accumulate, j-major OUT layout with streamed DMA.
"""

import numpy as np
import ml_dtypes

import concourse.bass as bass
import concourse.mybir as mybir
import concourse.tile as tile
from concourse import bacc
from concourse.bass_utils import run_bass_kernel_spmd
from concourse.masks import make_identity

BF = mybir.dt.bfloat16
F32 = mybir.dt.float32
F32R = mybir.dt.float32r

B, N, C, D, K = 16, 2048, 64, 10, 3
NCORES = 8
B2 = B // NCORES          # 2 batches per core
MT = N // 128             # 16 row tiles
C2 = 2 * C                # 128
OG = 2 * C                # gate output width 128
OU = C                    # update output width 64
WOG = OG * D              # 1280 (o,d)-interleaved gate width
WOU = OU * D              # 640
GCH = [(0, 510), (510, 510), (1020, 260)]   # gate (o,d) chunks, mult of 10
UCH = [(0, 510), (510, 130)]                # update chunks

_CACHE = {}


def _build():
    nc = bacc.Bacc("TRN2", target_bir_lowering=False, debug=False,
                   num_devices=NCORES)

    def inp(name, shape, dt):
        return nc.dram_tensor(name, list(shape), dt, kind="ExternalInput").ap()

    xc_d = inp("xc", (128, MT * B2 * C2), BF)    # [p, (m,b,c2)] xcat tiles
    ew_d = inp("ew", (D, N), F32R)               # E^T
    et_d = inp("et", (128, MT * D), F32)         # [p, (j,d)] E rows
    gw_d = inp("gw", (128, K * WOG), BF)         # [c2, (k,o,d)] gate pool
    uw_d = inp("uw", (128, K * WOU), BF)
    gb_d = inp("gb", (128, MT * OG), BF)         # [p, (j,o)] E@gate_bpool
    ub_d = inp("ub", (128, MT * OU), BF)
    lw_d = inp("lw", (C2, C), BF)                # lin_w^T
    lb_d = inp("lb", (128, C), F32)              # lin_b tiled
    out_d = nc.dram_tensor("out", [128, MT * B2 * C], F32,
                           kind="ExternalOutput").ap()   # [p, (j,b,c)]

    AFT = mybir.ActivationFunctionType
    MULT = mybir.AluOpType.mult
    ADD = mybir.AluOpType.add
    AXX = mybir.AxisListType.X

    with tile.TileContext(nc) as tc:
        with (
            tc.tile_pool(name="const", bufs=1) as const,
            tc.tile_pool(name="sraw", bufs=4) as srp,
            tc.tile_pool(name="stat", bufs=4) as stat,
            tc.tile_pool(name="xgp", bufs=4) as xgp,
            tc.tile_pool(name="scp", bufs=4) as scp,
            tc.tile_pool(name="accp", bufs=4) as accp,
            tc.tile_pool(name="sgp", bufs=4) as sgp,
            # PSUM: "big" 2 banks x2, "pd" 1x2, "pt" 1x2 = 8 banks
            tc.tile_pool(name="pB", bufs=2, space="PSUM") as pB,
            tc.tile_pool(name="pD", bufs=2, space="PSUM") as pD,
            tc.tile_pool(name="pT", bufs=2, space="PSUM") as pT,
        ):
            ident = const.tile([128, 128], BF)
            make_identity(nc, ident[:])

            # ---- HAM warmup: dense matmuls so the PE clock un-throttles
            # (K=4/8 -> 8/8) and stays there while the input DMAs land.
            # Small tensors (EW for EE^T) are DMA'd first so real PE work
            # starts within ~2us.
            wrm = const.tile([128, 512], BF)
            nc.gpsimd.memset(wrm[:], 0.0)

            EW = const.tile([D, N], F32R)
            nc.sync.dma_start(EW[:], ew_d[:])
            ET = const.tile([128, MT * D], F32)
            nc.sync.dma_start(ET[:], et_d[:])
            LW = const.tile([C2, C], BF)
            nc.sync.dma_start(LW[:], lw_d[:])
            LB = const.tile([128, C], F32)
            nc.sync.dma_start(LB[:], lb_d[:])
            GB = const.tile([128, MT * OG], BF)
            nc.sync.dma_start(GB[:], gb_d[:])
            UB = const.tile([128, MT * OU], BF)
            nc.sync.dma_start(UB[:], ub_d[:])
            XC = const.tile([128, MT * B2 * C2], BF)
            nc.sync.dma_start(XC[:], xc_d[:])
            GW = const.tile([128, K * WOG], BF)
            nc.sync.dma_start(GW[:], gw_d[:])
            UW = const.tile([128, K * WOU], BF)
            nc.sync.dma_start(UW[:], uw_d[:])

            for i in range(10):
                pwm = pB.tile([128, 1024], F32, tag="big", name="pwm")
                nc.tensor.matmul(pwm[:, 0:512], ident[:], wrm[:],
                                 start=True, stop=True)

            A = [const.tile([128, N], BF, tag=f"A{j}", name=f"A{j}")
                 for j in range(MT)]
            RINV = const.tile([128, MT], F32)
            OUT = const.tile([128, MT * B2 * C], F32)   # [p, (j,b,c)]

            # persistent diffusion state
            Y1 = const.tile([128, MT * B2 * C2], BF)    # rinv*(A@[x|s])
            U2 = const.tile([128, MT * B2 * C2], BF)    # rinv*(A@Y1)
            # stitched [x|zr], [y1x|y1z], [u2x|u2z] per (j, b): 64+64 cols
            XZ = const.tile([128, MT * B2 * C2], BF)
            YZ = const.tile([128, MT * B2 * C2], BF)
            UZ = const.tile([128, MT * B2 * C2], BF)

            def cat3(t, j):
                return t[:, j * 256:(j + 1) * 256] \
                    .rearrange("p (b c) -> p b c", b=B2)

            # partial accumulators for the split first hop (m 0..7 during
            # phase S, m 8..15 after)
            PRT = const.tile([128, MT * B2 * C2], BF)

            def d1a_passA(j):
                pd = pD.tile([128, B2 * C2], F32, tag="pd")
                for m in range(MT // 2):
                    nc.tensor.matmul(pd[:],
                                     A[m][:, j * 128:(j + 1) * 128],
                                     XC[:, m * 256:(m + 1) * 256],
                                     start=(m == 0), stop=(m == MT // 2 - 1))
                nc.scalar.activation(PRT[:, j * 256:(j + 1) * 256], pd[:],
                                     AFT.Copy, scale=RINV[:, j:j + 1])

            # ---- phase S: A = max(exp(E@E^T), 1), rinv = 1/rowsum ----
            # (exp(relu(x)) == max(exp(x), 1); clamp+rowsum fused on DVE).
            # EE^T lands in 2-bank PSUM tiles so exp evicts 1024 wide.
            # Once the first 8 A-tiles exist, the first half of the A@[x|s]
            # hop runs interleaved to keep the PE busy under S's DVE chain.
            for j in range(MT):
                etmp = srp.tile([128, N], BF, tag="etmp")
                for h in range(2):
                    ps = pB.tile([128, 1024], F32, tag="big", name="ps")
                    for q in range(2):
                        nc.tensor.matmul(ps[:, q * 512:(q + 1) * 512],
                                         EW[:, j * 128:(j + 1) * 128],
                                         EW[:, (2 * h + q) * 512:
                                             (2 * h + q + 1) * 512],
                                         start=True, stop=True)
                    nc.scalar.activation(etmp[:, h * 1024:(h + 1) * 1024],
                                         ps[:], AFT.Exp)
                zs = stat.tile([128, 1], F32, tag="zs")
                nc.vector.tensor_scalar(A[j][:], etmp[:], 1.0, 0.0,
                                        mybir.AluOpType.max,
                                        mybir.AluOpType.add,
                                        accum_out=zs[:])
                nc.vector.reciprocal(RINV[:, j:j + 1], zs[:])
                if j >= MT // 2:
                    d1a_passA(j - MT // 2)
                    # d1a alone is ~78% PE-busy which still re-throttles;
                    # filler holds the busy-window high
                    ptw = pT.tile([128, 384], F32, tag="pt", name="ptw")
                    for _ in range(3):
                        nc.tensor.matmul(ptw[:], ident[:], wrm[:, 0:384],
                                         start=True, stop=True)
                else:
                    # EE^T alone is ~35% PE-busy at this point and the HAM
                    # re-throttles the PE clock to 1.2GHz; ~2us of filler
                    # matmuls per iteration keeps the busy-window high so
                    # the whole first hop runs at 2.4GHz
                    for _ in range(2):
                        ptw = pT.tile([128, 384], F32, tag="pt", name="ptw")
                        for _ in range(6):
                            nc.tensor.matmul(ptw[:], ident[:],
                                             wrm[:, 0:384],
                                             start=True, stop=True)
            for j in range(MT // 2, MT):
                d1a_passA(j)

            # ---- diffusion hop: dst_j = rinv_j * (A @ rhs) ----
            def hop_j(j, rhs_fn, evict_fn, m0=0):
                pd = pD.tile([128, B2 * C2], F32, tag="pd")
                w = rhs_fn(0).free_size()
                for m in range(m0, MT):
                    nc.tensor.matmul(pd[:, 0:w],
                                     A[m][:, j * 128:(j + 1) * 128],
                                     rhs_fn(m),
                                     start=(m == m0), stop=(m == MT - 1))
                evict_fn(pd[:, 0:w])

            def full_evict(dst, j):
                def ev(pdw):
                    nc.scalar.activation(dst[:, j * 256:(j + 1) * 256], pdw,
                                         AFT.Copy, scale=RINV[:, j:j + 1])
                return ev

            def z_evict(dst, j, on_dve=False):
                # write z-halves into cols [64:128] of each 128-col group
                def ev(pdw):
                    if on_dve:
                        # during the zr-hop phase DVE is idle while the
                        # scalar eviction queue trails the PE by ~4us and
                        # stalls the next hop phase; evict on DVE instead
                        nc.vector.scalar_tensor_tensor(
                            cat3(dst, j)[:, :, C:C2],
                            pdw.rearrange("p (b c) -> p b c", b=B2),
                            RINV[:, j:j + 1],
                            wrm[:, 0:128].rearrange("p (b c) -> p b c",
                                                    b=B2),
                            MULT, ADD)
                    else:
                        nc.scalar.activation(
                            cat3(dst, j)[:, :, C:C2],
                            pdw.rearrange("p (b c) -> p b c", b=B2),
                            AFT.Copy, scale=RINV[:, j:j + 1])
                return ev

            for j in range(MT):
                # second half of the first hop; fused add of the pass-A
                # partial during eviction
                def evA(pdw, j=j):
                    nc.vector.scalar_tensor_tensor(
                        Y1[:, j * 256:(j + 1) * 256], pdw,
                        RINV[:, j:j + 1],
                        PRT[:, j * 256:(j + 1) * 256],
                        MULT, ADD)
                hop_j(j, lambda m: XC[:, m * 256:(m + 1) * 256], evA,
                      m0=MT // 2)
                # prefill x / y1x columns of the stitched tiles (gpsimd:
                # SBUF-to-SBUF, keeps scalar/DVE free)
                nc.gpsimd.tensor_copy(cat3(XZ, j)[:, :, 0:C],
                                      cat3(XC, j)[:, :, 0:C])
                nc.gpsimd.tensor_copy(cat3(YZ, j)[:, :, 0:C],
                                      cat3(Y1, j)[:, :, 0:C])

            # ---- weight application: prep (transposes) + main, software-
            # pipelined one j apart so the PE never heads-of-line blocks on
            # the cross-engine chain.  Both local batches are processed in
            # one batched PSUM tile / DVE op per chunk.
            def weight_prep(is_gate, j):
                srcs = (XC, Y1, U2) if is_gate else (XZ, YZ, UZ)
                pt = pT.tile([128, B2 * K * 128], BF, tag="pt")
                for b in range(B2):
                    for k, src in enumerate(srcs):
                        nc.tensor.transpose(
                            pt[:, (b * K + k) * 128:(b * K + k + 1) * 128],
                            src[:, j * 256 + b * 128: j * 256 + (b + 1) * 128],
                            ident[:])
                xgT = xgp.tile([128, B2, K, 128], BF, tag="xgT")
                nc.scalar.activation(xgT[:].rearrange("p b k c -> p (b k c)"),
                                     pt[:], AFT.Copy)
                return xgT

            def weight_main(is_gate, j, xgT):
                o = OG if is_gate else OU
                wsrc = GW if is_gate else UW
                wod = WOG if is_gate else WOU
                chunks = GCH if is_gate else UCH

                acc = accp.tile([128, B2, o], BF, tag=f"acc{o}")
                for ci, (q0, w) in enumerate(chunks):
                    pw = pB.tile([128, B2, 512], F32, tag="big", name="pw")
                    for b in range(B2):
                        for k in range(K):
                            nc.tensor.matmul(
                                pw[:, b, 0:w],
                                xgT[:, b, k, :],
                                wsrc[:, k * wod + q0: k * wod + q0 + w],
                                start=(k == 0), stop=(k == K - 1))
                    sc = scp.tile([128, B2, 512], BF, tag="sc")
                    e4 = ET[:, j * D:(j + 1) * D].unsqueeze(1).unsqueeze(1) \
                        .broadcast_to([128, B2, w // D, D])
                    nc.vector.tensor_tensor(
                        sc[:, :, 0:w].rearrange("p b (o d) -> p b o d", d=D),
                        pw[:, :, 0:w].rearrange("p b (o d) -> p b o d", d=D),
                        e4, MULT)
                    with nc.allow_low_precision(reason="d-sum, 10 terms"):
                        nc.vector.tensor_reduce(
                            acc[:, :, q0 // D: (q0 + w) // D],
                            sc[:, :, 0:w].rearrange("p b (o d) -> p b o d",
                                                    d=D),
                            AXX, ADD)
                if is_gate:
                    gbj = GB[:, j * OG:(j + 1) * OG].unsqueeze(1) \
                        .broadcast_to([128, B2, OG])
                    nc.gpsimd.tensor_tensor(acc[:], acc[:], gbj, ADD)
                    sig = sgp.tile([128, B2 * OG], BF, tag="sig")
                    nc.scalar.activation(sig[:], acc[:].rearrange(
                        "p b o -> p (b o)"), AFT.Sigmoid)
                    pts = pT.tile([128, B2 * K * 128], BF, tag="pt")
                    for b in range(B2):
                        nc.tensor.transpose(
                            pts[:, b * 128:(b + 1) * 128],
                            sig[:, b * OG:(b + 1) * OG], ident[:])
                    sigT = sgp.tile([128, B2, OG], BF, tag="sigT")
                    nc.scalar.activation(
                        sigT[:].rearrange("p b o -> p (b o)"),
                        pts[:, 0:B2 * OG], AFT.Copy)
                    # pt tag, not pd: a pd-tag pz2 makes the first zr-hop
                    # wait (WAR) on the last gate block's DVE drain,
                    # stalling the PE ~5us at the phase transition
                    pz2 = pT.tile([128, 384], F32, tag="pt", name="pz2")
                    pz2 = pz2[:, 0:B2 * C]
                    for b in range(B2):
                        nc.tensor.matmul(pz2[:, b * C:(b + 1) * C],
                                         sigT[:, b, :], LW[:],
                                         start=True, stop=True)
                    # z_r written straight into [x|zr] cols [64:128]
                    lbb = LB[:].unsqueeze(1).broadcast_to([128, B2, C])
                    nc.vector.tensor_tensor(
                        cat3(XZ, j)[:, :, C:C2],
                        pz2[:].rearrange("p (b c) -> p b c", b=B2),
                        lbb, ADD)
                else:
                    ubj = UB[:, j * OU:(j + 1) * OU].unsqueeze(1) \
                        .broadcast_to([128, B2, OU])
                    nc.gpsimd.tensor_tensor(acc[:], acc[:], ubj, ADD)
                    nc.scalar.activation(
                        OUT[:, j * (B2 * C):(j + 1) * (B2 * C)],
                        acc[:].rearrange("p b o -> p (b o)"), AFT.Tanh)

            # D1b interleaved with the gate weight phase: PE streams U2
            # matmuls while DVE drains the previous block's d-contraction
            prev = None
            for j in range(MT):
                hop_j(j, lambda m: Y1[:, m * 256:(m + 1) * 256],
                      full_evict(U2, j))
                nc.gpsimd.tensor_copy(cat3(UZ, j)[:, :, 0:C],
                                      cat3(U2, j)[:, :, 0:C])
                cur = weight_prep(True, j)
                if prev is not None:
                    weight_main(True, j - 1, prev)
                prev = cur
            weight_main(True, MT - 1, prev)

            def zr_rhs(m):
                return cat3(XZ, m)[:, :, C:C2]

            def y1z_rhs(m):
                return cat3(YZ, m)[:, :, C:C2]

            for j in range(MT):
                hop_j(j, zr_rhs, z_evict(YZ, j, on_dve=True))
            prev = None
            for j in range(MT):
                hop_j(j, y1z_rhs, z_evict(UZ, j))
                cur = weight_prep(False, j)
                if prev is not None:
                    weight_main(False, j - 1, prev)
                    lo = (j - 1) * B2 * C
                    nc.sync.dma_start(out_d[:, lo:lo + B2 * C],
                                      OUT[:, lo:lo + B2 * C])
                prev = cur
            weight_main(False, MT - 1, prev)
            lo = (MT - 1) * B2 * C
            nc.sync.dma_start(out_d[:, lo:], OUT[:, lo:])

    nc.compile()
    return nc


def kernel(x, state, node_embeddings, gate_wpool, gate_bpool,
           upd_wpool, upd_bpool, lin_w, lin_b):
    x = np.asarray(x, np.float32)
    state = np.asarray(state, np.float32)
    E = np.asarray(node_embeddings, np.float32)
    gw = np.asarray(gate_wpool, np.float32)
    gb = np.asarray(gate_bpool, np.float32)
    uw = np.asarray(upd_wpool, np.float32)
    ub = np.asarray(upd_bpool, np.float32)
    lw = np.asarray(lin_w, np.float32)
    lb = np.asarray(lin_b, np.float32)
    bf = ml_dtypes.bfloat16

    if "nc" not in _CACHE:
        _CACHE["nc"] = _build()
    nc = _CACHE["nc"]

    def fold_cheb(w):
        # w: [D, K, Ci, O] -> w0-w2, w1, 2*w2 then [c2, k, o, d] tiling
        wm = np.stack([w[:, 0] - w[:, 2], w[:, 1], 2.0 * w[:, 2]], axis=1)
        return np.ascontiguousarray(
            wm.transpose(2, 1, 3, 0).reshape(C2, -1)).astype(bf)

    gwr = fold_cheb(gw)                           # [128, 3840]
    uwr = fold_cheb(uw)                           # [128, 1920]
    gbf = (E @ gb).reshape(MT, 128, OG).transpose(1, 0, 2) \
        .reshape(128, MT * OG).astype(bf)
    ubf = (E @ ub).reshape(MT, 128, OU).transpose(1, 0, 2) \
        .reshape(128, MT * OU).astype(bf)
    etr = np.ascontiguousarray(E.T)               # [10, 2048] f32
    ett = E.reshape(MT, 128, D).transpose(1, 0, 2).reshape(128, MT * D)
    ett = np.ascontiguousarray(ett)
    lwT = np.ascontiguousarray(lw.T).astype(bf)   # [128, 64]
    lbt = np.ascontiguousarray(np.tile(lb[None, :], (128, 1)))

    xcat = np.concatenate([x, state], axis=-1)    # [16, 2048, 128] f32

    in_maps = []
    for r in range(NCORES):
        xcr = xcat[2 * r:2 * r + 2].reshape(B2, MT, 128, C2) \
            .transpose(2, 1, 0, 3).reshape(128, MT * B2 * C2)
        in_maps.append({
            "xc": np.ascontiguousarray(xcr).astype(bf),
            "ew": etr, "et": ett,
            "gw": gwr, "uw": uwr,
            "gb": gbf, "ub": ubf,
            "lw": lwT, "lb": lbt,
        })
    global _LAST_IN_MAPS
    _LAST_IN_MAPS = in_maps
    res = run_bass_kernel_spmd(nc, in_maps, core_ids=list(range(NCORES)))
    outs = []
    for r in range(NCORES):
        o = res.results[r]["out"]                  # [128, 16*2*64] (j,b,c)
        o = o.reshape(128, MT, B2, C).transpose(2, 1, 0, 3) \
            .reshape(B2, N, C)
        outs.append(o)
    return np.concatenate(outs, axis=0).astype(np.float32)


# revision 25
# speedup vs baseline: 1.2894x; 1.0017x over previous
"""AGCRNCell distributed Bass kernel for 8 TRN2 NeuronCores.

Batch-parallel: B=16 -> 2 batches/core, zero collectives.  Each core:
  A = exp(relu(E @ E^T))      (symmetric -> A^T = A, no transposes;
                               softmax normalization deferred: S@v =
                               rinv * (A@v), rinv applied at eviction)
  diffusion hops as dense bf16 matmuls over 128x128 tiles of A,
  Chebyshev term folded into the weight pools host-side:
      sum_k xg_k w_k = xg0 (w0-w2) + y1 w1 + u2 (2 w2),
      y1 = rinv*(A@x), u2 = rinv*(A@y1)
  per-node adaptive weights factored through the embedding dim D=10:
      out[n,o] = sum_d E[n,d] * (xg[n,:] @ wpool[d,:,(o)])
  with weight pools laid out (o,d)-interleaved so the d-contraction is
  one DVE tensor_tensor(mult, E broadcast) + one tensor_reduce(X) per
  chunk, batched over both local batches.

v2: HAM warmup burst (PE idles ~90us at 1.2GHz otherwise), fused
1024-wide exp eviction via a shared 2-bank PSUM tag, weight phase
batched over b (half the DVE/scalar instructions), bf16 d-sum
accumulate, j-major OUT layout with streamed DMA.
"""

import numpy as np
import ml_dtypes

import concourse.bass as bass
import concourse.mybir as mybir
import concourse.tile as tile
from concourse import bacc
from concourse.bass_utils import run_bass_kernel_spmd
from concourse.masks import make_identity

BF = mybir.dt.bfloat16
F32 = mybir.dt.float32
F32R = mybir.dt.float32r

B, N, C, D, K = 16, 2048, 64, 10, 3
NCORES = 8
B2 = B // NCORES          # 2 batches per core
MT = N // 128             # 16 row tiles
C2 = 2 * C                # 128
OG = 2 * C                # gate output width 128
OU = C                    # update output width 64
WOG = OG * D              # 1280 (o,d)-interleaved gate width
WOU = OU * D              # 640
GCH = [(0, 510), (510, 510), (1020, 260)]   # gate (o,d) chunks, mult of 10
UCH = [(0, 510), (510, 130)]                # update chunks

_CACHE = {}


def _build():
    nc = bacc.Bacc("TRN2", target_bir_lowering=False, debug=False,
                   num_devices=NCORES)

    def inp(name, shape, dt):
        return nc.dram_tensor(name, list(shape), dt, kind="ExternalInput").ap()

    xc_d = inp("xc", (128, MT * B2 * C2), BF)    # [p, (m,b,c2)] xcat tiles
    ew_d = inp("ew", (D, N), F32R)               # E^T
    et_d = inp("et", (128, MT * D), F32)         # [p, (j,d)] E rows
    gw_d = inp("gw", (128, K * WOG), BF)         # [c2, (k,o,d)] gate pool
    uw_d = inp("uw", (128, K * WOU), BF)
    gb_d = inp("gb", (128, MT * OG), BF)         # [p, (j,o)] E@gate_bpool
    ub_d = inp("ub", (128, MT * OU), BF)
    lw_d = inp("lw", (C2, C), BF)                # lin_w^T
    lb_d = inp("lb", (128, C), F32)              # lin_b tiled
    out_d = nc.dram_tensor("out", [128, MT * B2 * C], F32,
                           kind="ExternalOutput").ap()   # [p, (j,b,c)]

    AFT = mybir.ActivationFunctionType
    MULT = mybir.AluOpType.mult
    ADD = mybir.AluOpType.add
    AXX = mybir.AxisListType.X

    with tile.TileContext(nc) as tc:
        with (
            tc.tile_pool(name="const", bufs=1) as const,
            tc.tile_pool(name="sraw", bufs=4) as srp,
            tc.tile_pool(name="stat", bufs=4) as stat,
            tc.tile_pool(name="xgp", bufs=4) as xgp,
            tc.tile_pool(name="scp", bufs=4) as scp,
            tc.tile_pool(name="accp", bufs=4) as accp,
            tc.tile_pool(name="sgp", bufs=4) as sgp,
            # PSUM: "big" 2 banks x2, "pd" 1x2, "pt" 1x2 = 8 banks
            tc.tile_pool(name="pB", bufs=2, space="PSUM") as pB,
            tc.tile_pool(name="pD", bufs=2, space="PSUM") as pD,
            tc.tile_pool(name="pT", bufs=2, space="PSUM") as pT,
        ):
            ident = const.tile([128, 128], BF)
            make_identity(nc, ident[:])

            # ---- HAM warmup: dense matmuls so the PE clock un-throttles
            # (K=4/8 -> 8/8) and stays there while the input DMAs land.
            # Small tensors (EW for EE^T) are DMA'd first so real PE work
            # starts within ~2us.
            wrm = const.tile([128, 512], BF)
            nc.gpsimd.memset(wrm[:], 0.0)

            EW = const.tile([D, N], F32R)
            nc.sync.dma_start(EW[:], ew_d[:])
            ET = const.tile([128, MT * D], F32)
            nc.sync.dma_start(ET[:], et_d[:])
            LW = const.tile([C2, C], BF)
            nc.sync.dma_start(LW[:], lw_d[:])
            LB = const.tile([128, C], F32)
            nc.sync.dma_start(LB[:], lb_d[:])
            GB = const.tile([128, MT * OG], BF)
            nc.sync.dma_start(GB[:], gb_d[:])
            UB = const.tile([128, MT * OU], BF)
            nc.sync.dma_start(UB[:], ub_d[:])
            XC = const.tile([128, MT * B2 * C2], BF)
            nc.sync.dma_start(XC[:], xc_d[:])
            GW = const.tile([128, K * WOG], BF)
            nc.sync.dma_start(GW[:], gw_d[:])
            UW = const.tile([128, K * WOU], BF)
            nc.sync.dma_start(UW[:], uw_d[:])

            for i in range(10):
                pwm = pB.tile([128, 1024], F32, tag="big", name="pwm")
                nc.tensor.matmul(pwm[:, 0:512], ident[:], wrm[:],
                                 start=True, stop=True)

            A = [const.tile([128, N], BF, tag=f"A{j}", name=f"A{j}")
                 for j in range(MT)]
            RINV = const.tile([128, MT], F32)
            OUT = const.tile([128, MT * B2 * C], F32)   # [p, (j,b,c)]

            # persistent diffusion state
            Y1 = const.tile([128, MT * B2 * C2], BF)    # rinv*(A@[x|s])
            U2 = const.tile([128, MT * B2 * C2], BF)    # rinv*(A@Y1)
            # stitched [x|zr], [y1x|y1z], [u2x|u2z] per (j, b): 64+64 cols
            XZ = const.tile([128, MT * B2 * C2], BF)
            YZ = const.tile([128, MT * B2 * C2], BF)
            UZ = const.tile([128, MT * B2 * C2], BF)

            def cat3(t, j):
                return t[:, j * 256:(j + 1) * 256] \
                    .rearrange("p (b c) -> p b c", b=B2)

            # partial accumulators for the split first hop (m 0..7 during
            # phase S, m 8..15 after)
            PRT = const.tile([128, MT * B2 * C2], BF)

            def d1a_passA(j):
                pd = pD.tile([128, B2 * C2], F32, tag="pd")
                for m in range(MT // 2):
                    nc.tensor.matmul(pd[:],
                                     A[m][:, j * 128:(j + 1) * 128],
                                     XC[:, m * 256:(m + 1) * 256],
                                     start=(m == 0), stop=(m == MT // 2 - 1))
                nc.scalar.activation(PRT[:, j * 256:(j + 1) * 256], pd[:],
                                     AFT.Copy, scale=RINV[:, j:j + 1])

            # ---- phase S: A = max(exp(E@E^T), 1), rinv = 1/rowsum ----
            # (exp(relu(x)) == max(exp(x), 1); clamp+rowsum fused on DVE).
            # EE^T lands in 2-bank PSUM tiles so exp evicts 1024 wide.
            # Once the first 8 A-tiles exist, the first half of the A@[x|s]
            # hop runs interleaved to keep the PE busy under S's DVE chain.
            for j in range(MT):
                etmp = srp.tile([128, N], BF, tag="etmp")
                for h in range(2):
                    ps = pB.tile([128, 1024], F32, tag="big", name="ps")
                    for q in range(2):
                        nc.tensor.matmul(ps[:, q * 512:(q + 1) * 512],
                                         EW[:, j * 128:(j + 1) * 128],
                                         EW[:, (2 * h + q) * 512:
                                             (2 * h + q + 1) * 512],
                                         start=True, stop=True)
                    nc.scalar.activation(etmp[:, h * 1024:(h + 1) * 1024],
                                         ps[:], AFT.Exp)
                zs = stat.tile([128, 1], F32, tag="zs")
                nc.vector.tensor_scalar(A[j][:], etmp[:], 1.0, 0.0,
                                        mybir.AluOpType.max,
                                        mybir.AluOpType.add,
                                        accum_out=zs[:])
                nc.vector.reciprocal(RINV[:, j:j + 1], zs[:])
                if j >= MT // 2:
                    d1a_passA(j - MT // 2)
                    # d1a alone is ~78% PE-busy which still re-throttles;
                    # filler holds the busy-window high
                    ptw = pT.tile([128, 384], F32, tag="pt", name="ptw")
                    for _ in range(3):
                        nc.tensor.matmul(ptw[:], ident[:], wrm[:, 0:384],
                                         start=True, stop=True)
                else:
                    # EE^T alone is ~35% PE-busy at this point and the HAM
                    # re-throttles the PE clock to 1.2GHz; ~2us of filler
                    # matmuls per iteration keeps the busy-window high so
                    # the whole first hop runs at 2.4GHz
                    for _ in range(2):
                        ptw = pT.tile([128, 384], F32, tag="pt", name="ptw")
                        for _ in range(6):
                            nc.tensor.matmul(ptw[:], ident[:],
                                             wrm[:, 0:384],
                                             start=True, stop=True)
            for j in range(MT // 2, MT):
                d1a_passA(j)

            # ---- diffusion hop: dst_j = rinv_j * (A @ rhs) ----
            def hop_j(j, rhs_fn, evict_fn, m0=0):
                pd = pD.tile([128, B2 * C2], F32, tag="pd")
                w = rhs_fn(0).free_size()
                for m in range(m0, MT):
                    nc.tensor.matmul(pd[:, 0:w],
                                     A[m][:, j * 128:(j + 1) * 128],
                                     rhs_fn(m),
                                     start=(m == m0), stop=(m == MT - 1))
                evict_fn(pd[:, 0:w])

            def full_evict(dst, j):
                def ev(pdw):
                    nc.scalar.activation(dst[:, j * 256:(j + 1) * 256], pdw,
                                         AFT.Copy, scale=RINV[:, j:j + 1])
                return ev

            def z_evict(dst, j, on_dve=False):
                # write z-halves into cols [64:128] of each 128-col group
                def ev(pdw):
                    if on_dve:
                        # during the zr-hop phase DVE is idle while the
                        # scalar eviction queue trails the PE by ~4us and
                        # stalls the next hop phase; evict on DVE instead
                        nc.vector.scalar_tensor_tensor(
                            cat3(dst, j)[:, :, C:C2],
                            pdw.rearrange("p (b c) -> p b c", b=B2),
                            RINV[:, j:j + 1],
                            wrm[:, 0:128].rearrange("p (b c) -> p b c",
                                                    b=B2),
                            MULT, ADD)
                    else:
                        nc.scalar.activation(
                            cat3(dst, j)[:, :, C:C2],
                            pdw.rearrange("p (b c) -> p b c", b=B2),
                            AFT.Copy, scale=RINV[:, j:j + 1])
                return ev

            for j in range(MT):
                # second half of the first hop; fused add of the pass-A
                # partial during eviction
                def evA(pdw, j=j):
                    nc.vector.scalar_tensor_tensor(
                        Y1[:, j * 256:(j + 1) * 256], pdw,
                        RINV[:, j:j + 1],
                        PRT[:, j * 256:(j + 1) * 256],
                        MULT, ADD)
                hop_j(j, lambda m: XC[:, m * 256:(m + 1) * 256], evA,
                      m0=MT // 2)
                # prefill x / y1x columns of the stitched tiles (gpsimd:
                # SBUF-to-SBUF, keeps scalar/DVE free)
                nc.gpsimd.tensor_copy(cat3(XZ, j)[:, :, 0:C],
                                      cat3(XC, j)[:, :, 0:C])
                nc.gpsimd.tensor_copy(cat3(YZ, j)[:, :, 0:C],
                                      cat3(Y1, j)[:, :, 0:C])

            # ---- weight application: prep (transposes) + main, software-
            # pipelined one j apart so the PE never heads-of-line blocks on
            # the cross-engine chain.  Both local batches are processed in
            # one batched PSUM tile / DVE op per chunk.
            def weight_prep(is_gate, j):
                srcs = (XC, Y1, U2) if is_gate else (XZ, YZ, UZ)
                pt = pT.tile([128, B2 * K * 128], BF, tag="pt")
                for b in range(B2):
                    for k, src in enumerate(srcs):
                        nc.tensor.transpose(
                            pt[:, (b * K + k) * 128:(b * K + k + 1) * 128],
                            src[:, j * 256 + b * 128: j * 256 + (b + 1) * 128],
                            ident[:])
                xgT = xgp.tile([128, B2, K, 128], BF, tag="xgT")
                nc.scalar.activation(xgT[:].rearrange("p b k c -> p (b k c)"),
                                     pt[:], AFT.Copy)
                return xgT

            def weight_main(is_gate, j, xgT):
                o = OG if is_gate else OU
                wsrc = GW if is_gate else UW
                wod = WOG if is_gate else WOU
                chunks = GCH if is_gate else UCH

                acc = accp.tile([128, B2, o], BF, tag=f"acc{o}")
                for ci, (q0, w) in enumerate(chunks):
                    pw = pB.tile([128, B2, 512], F32, tag="big", name="pw")
                    for b in range(B2):
                        for k in range(K):
                            nc.tensor.matmul(
                                pw[:, b, 0:w],
                                xgT[:, b, k, :],
                                wsrc[:, k * wod + q0: k * wod + q0 + w],
                                start=(k == 0), stop=(k == K - 1))
                    sc = scp.tile([128, B2, 512], BF, tag="sc")
                    e4 = ET[:, j * D:(j + 1) * D].unsqueeze(1).unsqueeze(1) \
                        .broadcast_to([128, B2, w // D, D])
                    nc.vector.tensor_tensor(
                        sc[:, :, 0:w].rearrange("p b (o d) -> p b o d", d=D),
                        pw[:, :, 0:w].rearrange("p b (o d) -> p b o d", d=D),
                        e4, MULT)
                    with nc.allow_low_precision(reason="d-sum, 10 terms"):
                        nc.vector.tensor_reduce(
                            acc[:, :, q0 // D: (q0 + w) // D],
                            sc[:, :, 0:w].rearrange("p b (o d) -> p b o d",
                                                    d=D),
                            AXX, ADD)
                if is_gate:
                    gbj = GB[:, j * OG:(j + 1) * OG].unsqueeze(1) \
                        .broadcast_to([128, B2, OG])
                    nc.gpsimd.tensor_tensor(acc[:], acc[:], gbj, ADD)
                    sig = sgp.tile([128, B2 * OG], BF, tag="sig")
                    nc.scalar.activation(sig[:], acc[:].rearrange(
                        "p b o -> p (b o)"), AFT.Sigmoid)
                    pts = pT.tile([128, B2 * K * 128], BF, tag="pt")
                    for b in range(B2):
                        nc.tensor.transpose(
                            pts[:, b * 128:(b + 1) * 128],
                            sig[:, b * OG:(b + 1) * OG], ident[:])
                    sigT = sgp.tile([128, B2, OG], BF, tag="sigT")
                    nc.scalar.activation(
                        sigT[:].rearrange("p b o -> p (b o)"),
                        pts[:, 0:B2 * OG], AFT.Copy)
                    # pt tag, not pd: a pd-tag pz2 makes the first zr-hop
                    # wait (WAR) on the last gate block's DVE drain,
                    # stalling the PE ~5us at the phase transition
                    pz2 = pT.tile([128, 384], F32, tag="pt", name="pz2")
                    pz2 = pz2[:, 0:B2 * C]
                    for b in range(B2):
                        nc.tensor.matmul(pz2[:, b * C:(b + 1) * C],
                                         sigT[:, b, :], LW[:],
                                         start=True, stop=True)
                    # z_r written straight into [x|zr] cols [64:128]
                    lbb = LB[:].unsqueeze(1).broadcast_to([128, B2, C])
                    nc.vector.tensor_tensor(
                        cat3(XZ, j)[:, :, C:C2],
                        pz2[:].rearrange("p (b c) -> p b c", b=B2),
                        lbb, ADD)
                else:
                    ubj = UB[:, j * OU:(j + 1) * OU].unsqueeze(1) \
                        .broadcast_to([128, B2, OU])
                    nc.gpsimd.tensor_tensor(acc[:], acc[:], ubj, ADD)
                    nc.scalar.activation(
                        OUT[:, j * (B2 * C):(j + 1) * (B2 * C)],
                        acc[:].rearrange("p b o -> p (b o)"), AFT.Tanh)

            # D1b interleaved with the gate weight phase: PE streams U2
            # matmuls while DVE drains the previous block's d-contraction
            prev = None
            for j in range(MT):
                hop_j(j, lambda m: Y1[:, m * 256:(m + 1) * 256],
                      full_evict(U2, j))
                nc.gpsimd.tensor_copy(cat3(UZ, j)[:, :, 0:C],
                                      cat3(U2, j)[:, :, 0:C])
                cur = weight_prep(True, j)
                if prev is not None:
                    weight_main(True, j - 1, prev)
                prev = cur
            weight_main(True, MT - 1, prev)

            def zr_rhs(m):
                return cat3(XZ, m)[:, :, C:C2]

            def y1z_rhs(m):
                return cat3(YZ, m)[:, :, C:C2]

            for j in range(MT):
                hop_j(j, zr_rhs, z_evict(YZ, j, on_dve=True))
                # the 128-col zr hops are LDW-bound at ~78% PE busy, which
                # lets the HAM re-throttle the clock mid-phase; filler
                # holds it at 8/8
                ptw2 = pT.tile([128, 384], F32, tag="pt", name="ptw2")
                nc.tensor.matmul(ptw2[:], ident[:], wrm[:, 0:384],
                                 start=True, stop=True)
            prev = None
            for j in range(MT):
                hop_j(j, y1z_rhs, z_evict(UZ, j))
                cur = weight_prep(False, j)
                if prev is not None:
                    weight_main(False, j - 1, prev)
                    lo = (j - 1) * B2 * C
                    nc.sync.dma_start(out_d[:, lo:lo + B2 * C],
                                      OUT[:, lo:lo + B2 * C])
                prev = cur
            weight_main(False, MT - 1, prev)
            lo = (MT - 1) * B2 * C
            nc.sync.dma_start(out_d[:, lo:], OUT[:, lo:])

    nc.compile()
    return nc


def kernel(x, state, node_embeddings, gate_wpool, gate_bpool,
           upd_wpool, upd_bpool, lin_w, lin_b):
    x = np.asarray(x, np.float32)
    state = np.asarray(state, np.float32)
    E = np.asarray(node_embeddings, np.float32)
    gw = np.asarray(gate_wpool, np.float32)
    gb = np.asarray(gate_bpool, np.float32)
    uw = np.asarray(upd_wpool, np.float32)
    ub = np.asarray(upd_bpool, np.float32)
    lw = np.asarray(lin_w, np.float32)
    lb = np.asarray(lin_b, np.float32)
    bf = ml_dtypes.bfloat16

    if "nc" not in _CACHE:
        _CACHE["nc"] = _build()
    nc = _CACHE["nc"]

    def fold_cheb(w):
        # w: [D, K, Ci, O] -> w0-w2, w1, 2*w2 then [c2, k, o, d] tiling
        wm = np.stack([w[:, 0] - w[:, 2], w[:, 1], 2.0 * w[:, 2]], axis=1)
        return np.ascontiguousarray(
            wm.transpose(2, 1, 3, 0).reshape(C2, -1)).astype(bf)

    gwr = fold_cheb(gw)                           # [128, 3840]
    uwr = fold_cheb(uw)                           # [128, 1920]
    gbf = (E @ gb).reshape(MT, 128, OG).transpose(1, 0, 2) \
        .reshape(128, MT * OG).astype(bf)
    ubf = (E @ ub).reshape(MT, 128, OU).transpose(1, 0, 2) \
        .reshape(128, MT * OU).astype(bf)
    etr = np.ascontiguousarray(E.T)               # [10, 2048] f32
    ett = E.reshape(MT, 128, D).transpose(1, 0, 2).reshape(128, MT * D)
    ett = np.ascontiguousarray(ett)
    lwT = np.ascontiguousarray(lw.T).astype(bf)   # [128, 64]
    lbt = np.ascontiguousarray(np.tile(lb[None, :], (128, 1)))

    xcat = np.concatenate([x, state], axis=-1)    # [16, 2048, 128] f32

    in_maps = []
    for r in range(NCORES):
        xcr = xcat[2 * r:2 * r + 2].reshape(B2, MT, 128, C2) \
            .transpose(2, 1, 0, 3).reshape(128, MT * B2 * C2)
        in_maps.append({
            "xc": np.ascontiguousarray(xcr).astype(bf),
            "ew": etr, "et": ett,
            "gw": gwr, "uw": uwr,
            "gb": gbf, "ub": ubf,
            "lw": lwT, "lb": lbt,
        })
    global _LAST_IN_MAPS
    _LAST_IN_MAPS = in_maps
    res = run_bass_kernel_spmd(nc, in_maps, core_ids=list(range(NCORES)))
    outs = []
    for r in range(NCORES):
        o = res.results[r]["out"]                  # [128, 16*2*64] (j,b,c)
        o = o.reshape(128, MT, B2, C).transpose(2, 1, 0, 3) \
            .reshape(B2, N, C)
        outs.append(o)
    return np.concatenate(outs, axis=0).astype(np.float32)
